# revision 3
# baseline (speedup 1.0000x reference)
"""Trainium2 Bass kernel for nn_CLFMv2_NoTemporalEmb (graph-PDE message passing).

Strategy: data-parallel over batch B=64 across 8 NeuronCores (8 batches/core).
Per core, activations are stored "pair-packed feature-major":
    tensor[psi, tau],  psi = (batch_parity)*64 + d  (128 partitions),
                       tau = (batch_pair)*1024 + node  (1024 per pair tensor).
Every weight matmul uses block-diagonal [128,128] stationary operands so all
matmuls have K=128, M=128 and PSUM dst partition 0 (required for float32r).
The Laplacian A@field runs with PE-transposed field tiles as the stationary
operand against a precomputed, row-softmaxed, alpha*dt-scaled adjacency
transpose; the (1 - alpha*dt)*field residual is folded into its diagonal.
All matmuls use float32r (full PE rate, ~1e-4 relative rounding).
"""

import numpy as np

import concourse.bacc as bacc
import concourse.tile as tile
import concourse.mybir as mybir
from concourse.bass_utils import run_bass_kernel_spmd

F32 = mybir.dt.float32
F32R = mybir.dt.float32r
AF = mybir.ActivationFunctionType
ALU = mybir.AluOpType

B, L, N, D, H, O = 64, 12, 1024, 64, 128, 12
STEPS = 4
NCORES = 8
BL = B // NCORES          # 8 batches per core
PAIRS = BL // 2           # 4
KCH = N // 128            # 8 adjacency chunks


def _build(alpha: float):
    dt_ = 1.0 / STEPS
    c_lap = alpha * dt_

    nc = bacc.Bacc("TRN2", target_bir_lowering=False, debug=False)

    def din(name, shape, dtype=F32R):
        return nc.dram_tensor(name, shape, dtype, kind="ExternalInput")

    hist = din("hist", [BL, L, N])
    adj = din("adj", [N, N], F32)
    w1eA = din("w1eA", [2 * L, H])
    w1eB = din("w1eB", [2 * L, H])
    w2eA = din("w2eA", [H, 2 * D])
    w2eB = din("w2eB", [H, 2 * D])
    pw1A = din("pw1A", [2 * D, H])
    pw1B = din("pw1B", [2 * D, H])
    pw2A = din("pw2A", [H, 2 * D])
    pw2B = din("pw2B", [H, 2 * D])
    wzbd = din("wzbd", [2 * D, 2 * D])
    uzbd = din("uzbd", [2 * D, 2 * D])
    whbd = din("whbd", [2 * D, 2 * D])
    uhbd = din("uhbd", [2 * D, 2 * D])
    wobd = din("wobd", [2 * D, 2 * D])
    dw1A = din("dw1A", [2 * D, H])
    dw1B = din("dw1B", [2 * D, H])
    dw2A = din("dw2A", [H, 2 * O])
    dw2B = din("dw2B", [H, 2 * O])
    ieye = din("ieye", [128, 128])     # identity (transposes, +fe fold)
    c0I = din("c0I", [128, 128])       # (1-alpha*dt)*I for adjacency diag

    bias_names = ["eb1A", "eb1B", "eb2", "pb1A", "pb1B", "pb2",
                  "bz", "bh", "bo", "db1A", "db1B"]
    biases = {n: din(n, [128, 1], F32) for n in bias_names}
    biases["db2"] = din("db2", [2 * O, 1], F32)

    out = nc.dram_tensor("out", [BL, O, N], F32, kind="ExternalOutput")

    with tile.TileContext(nc) as tc:
        import contextlib
        with contextlib.ExitStack() as ctx:
            pp = ctx.enter_context(tc.tile_pool(name="persist", bufs=1))
            hab = ctx.enter_context(tc.tile_pool(name="hab", bufs=4))
            ftp = ctx.enter_context(tc.tile_pool(name="ftp", bufs=2))
            tmp = ctx.enter_context(tc.tile_pool(name="tmp", bufs=2))
            ahp = ctx.enter_context(tc.tile_pool(name="ahp", bufs=2))
            adjp = ctx.enter_context(tc.tile_pool(name="adjp", bufs=2))
            smp = ctx.enter_context(tc.tile_pool(name="smp", bufs=4))
            fep = ctx.enter_context(tc.tile_pool(name="fep", bufs=2))
            zcp = ctx.enter_context(tc.tile_pool(name="zcp", bufs=4))
            x2p = ctx.enter_context(tc.tile_pool(name="x2p", bufs=2))
            o2p = ctx.enter_context(tc.tile_pool(name="o2p", bufs=2))
            psA = ctx.enter_context(tc.tile_pool(name="psA", bufs=2, space="PSUM"))
            psB = ctx.enter_context(tc.tile_pool(name="psB", bufs=2, space="PSUM"))

            # ---- load weights/biases into SBUF ----
            wt = {}
            for name, hnd in [("w1eA", w1eA), ("w1eB", w1eB)]:
                t = pp.tile([2 * L, H], F32R, tag=name, name=name)
                nc.sync.dma_start(t[:], hnd[:, :])
                wt[name] = t
            for name, hnd in [("w2eA", w2eA), ("w2eB", w2eB),
                              ("pw1A", pw1A), ("pw1B", pw1B),
                              ("pw2A", pw2A), ("pw2B", pw2B),
                              ("wzbd", wzbd), ("uzbd", uzbd),
                              ("whbd", whbd), ("uhbd", uhbd),
                              ("wobd", wobd),
                              ("dw1A", dw1A), ("dw1B", dw1B),
                              ("ieye", ieye), ("c0I", c0I)]:
                t = pp.tile([128, 128], F32R, tag=name, name=name)
                nc.sync.dma_start(t[:], hnd[:, :])
                wt[name] = t
            for name, hnd in [("dw2A", dw2A), ("dw2B", dw2B)]:
                t = pp.tile([H, 2 * O], F32R, tag=name, name=name)
                nc.sync.dma_start(t[:], hnd[:, :])
                wt[name] = t
            bs = {}
            for name, hnd in biases.items():
                t = pp.tile([hnd.shape[0], 1], F32, tag="b_" + name, name="b_" + name)
                nc.sync.dma_start(t[:], hnd[:, :])
                bs[name] = t

            # ---- adjacency: row softmax (no max-sub; logits are tiny),
            #      scale by alpha*dt/rowsum, add (1-alpha*dt)I, transpose ----
            AT = pp.tile([128, KCH * N], F32R, tag="AT", name="AT")  # [m', (k, n)]
            for c in range(KCH):
                ac = adjp.tile([128, N], F32, tag="adj", name="ac")
                nc.sync.dma_start(ac[:], adj[c * 128:(c + 1) * 128, :])
                rs = smp.tile([128, 1], F32, tag="small", name="rs")
                nc.scalar.activation(ac[:], ac[:], AF.Exp, accum_out=rs[:])
                rr = smp.tile([128, 1], F32, tag="small", name="rr")
                nc.vector.reciprocal(rr[:], rs[:])
                ah = ahp.tile([128, N], F32R, tag="ah", name="ah")
                nc.vector.tensor_scalar(ah[:], ac[:], rr[:, 0:1], c_lap,
                                        ALU.mult, ALU.mult)
                # diagonal fold: rows c*128..c*128+127 own diag block k==c
                nc.vector.tensor_tensor(ah[:, c * 128:(c + 1) * 128],
                                        ah[:, c * 128:(c + 1) * 128],
                                        wt["c0I"][:], ALU.add)
                pt = psA.tile([128, N], F32R, tag="psA", name="psat")
                for k in range(KCH):
                    nc.tensor.transpose(pt[:, k * 128:(k + 1) * 128],
                                        ah[:, k * 128:(k + 1) * 128],
                                        wt["ieye"][:])
                # strided evac: block (c,k) -> AT[:, k*1024 + c*128]
                nc.vector.tensor_copy(
                    AT[:].rearrange("p (k n) -> p k n", k=KCH)[:, :, c * 128:(c + 1) * 128],
                    pt[:].rearrange("p (k n) -> p k n", k=KCH),
                )

            # per-pair persistent activations
            field = [pp.tile([128, N], F32R, tag=f"field{p}", name=f"field{p}") for p in range(PAIRS)]
            state = [pp.tile([128, N], F32R, tag=f"state{p}", name=f"state{p}") for p in range(PAIRS)]

            # ---- encoder ----
            for p in range(PAIRS):
                # x pair-packed: xp[s*12+l, n] = hist[2p+s, l, n]
                xp = x2p.tile([2 * L, N], F32R, tag="x2p", name="xp")
                nc.sync.dma_start(xp[0:L, :], hist[2 * p, :, :])
                nc.sync.dma_start(xp[L:2 * L, :], hist[2 * p + 1, :, :])
                hea = hab.tile([128, N], F32R, tag="hab", name="hea")
                heb = hab.tile([128, N], F32R, tag="hab", name="heb")
                for (wname, bname, dst) in [("w1eA", "eb1A", hea),
                                            ("w1eB", "eb1B", heb)]:
                    ph = psA.tile([128, N], F32, tag="psA", name="psah")
                    for hf in range(2):
                        sl = slice(hf * 512, (hf + 1) * 512)
                        nc.tensor.matmul(ph[:, sl], wt[wname][:], xp[:, sl],
                                         start=True, stop=True)
                    nc.scalar.activation(dst[:], ph[:], AF.Relu, bias=bs[bname][:])
                pf = psB.tile([128, N], F32, tag="psB", name="psbf")
                for hf in range(2):
                    sl = slice(hf * 512, (hf + 1) * 512)
                    nc.tensor.matmul(pf[:, sl], wt["w2eA"][:], hea[:, sl],
                                     start=True, stop=False)
                    nc.tensor.matmul(pf[:, sl], wt["w2eB"][:], heb[:, sl],
                                     start=False, stop=True)
                nc.scalar.activation(field[p][:], pf[:], AF.Identity,
                                     bias=bs["eb2"][:])

            # ---- main steps ----
            for s in range(STEPS):
                first = (s == 0)
                for p in range(PAIRS):
                    # A) transpose field pair -> fieldT [m', (k, psi)]
                    ptr = psA.tile([128, N], F32R, tag="psA", name="psatr")
                    for k in range(KCH):
                        nc.tensor.transpose(ptr[:, k * 128:(k + 1) * 128],
                                            field[p][:, k * 128:(k + 1) * 128],
                                            wt["ieye"][:])
                    ft = ftp.tile([128, N], F32R, tag="ft", name="ft")
                    nc.vector.tensor_copy(ft[:], ptr[:])

                    # B) pde layer 1: hA/hB = tanh(field @ w1 + b1)
                    ha = hab.tile([128, N], F32R, tag="hab", name="ha")
                    hb = hab.tile([128, N], F32R, tag="hab", name="hb")
                    for (wname, bname, dst) in [("pw1A", "pb1A", ha),
                                                ("pw1B", "pb1B", hb)]:
                        ph = psA.tile([128, N], F32, tag="psA", name="psah")
                        for hf in range(2):
                            sl = slice(hf * 512, (hf + 1) * 512)
                            nc.tensor.matmul(ph[:, sl], wt[wname][:],
                                             field[p][:, sl],
                                             start=True, stop=True)
                        nc.scalar.activation(dst[:], ph[:], AF.Tanh,
                                             bias=bs[bname][:])

                    # C) fe psum: Laplacian(+c0*field fold) + pde layer 2
                    fe_t = fep.tile([128, N], F32R, tag="fe", name="fe_t")
                    z_t = zcp.tile([128, N], F32, tag="zc", name="z_t")
                    c_t = zcp.tile([128, N], F32, tag="zc", name="c_t")
                    pfe = psB.tile([128, N], F32, tag="psB", name="psbfe")
                    for hf in range(2):
                        sl = slice(hf * 512, (hf + 1) * 512)
                        for k in range(KCH):
                            nc.tensor.matmul(
                                pfe[:, sl],
                                ft[:, k * 128:(k + 1) * 128],
                                AT[:, k * N + hf * 512:k * N + (hf + 1) * 512],
                                start=(k == 0), stop=False)
                        nc.tensor.matmul(pfe[:, sl], wt["pw2A"][:], ha[:, sl],
                                         start=False, stop=False)
                        nc.tensor.matmul(pfe[:, sl], wt["pw2B"][:], hb[:, sl],
                                         start=False, stop=True)
                    nc.scalar.activation(fe_t[:], pfe[:], AF.Identity,
                                         bias=bs["pb2"][:])

                    # D) GRU gates: z, cand
                    for (wname, uname, bname, func, dst) in [
                        ("wzbd", "uzbd", "bz", AF.Sigmoid, z_t),
                        ("whbd", "uhbd", "bh", AF.Tanh, c_t),
                    ]:
                        pz = psB.tile([128, N], F32, tag="psB", name="psbz")
                        for hf in range(2):
                            sl = slice(hf * 512, (hf + 1) * 512)
                            nc.tensor.matmul(pz[:, sl], wt[wname][:], fe_t[:, sl],
                                             start=True, stop=first)
                            if not first:
                                nc.tensor.matmul(pz[:, sl], wt[uname][:],
                                                 state[p][:, sl],
                                                 start=False, stop=True)
                        nc.scalar.activation(dst[:], pz[:], func, bias=bs[bname][:])

                    # E) state update
                    if first:
                        nc.vector.tensor_tensor(state[p][:], z_t[:], c_t[:],
                                                ALU.mult)
                    else:
                        t1 = tmp.tile([128, N], F32, tag="tmp", name="t1")
                        nc.vector.tensor_tensor(t1[:], c_t[:], state[p][:],
                                                ALU.subtract)
                        nc.vector.tensor_tensor(t1[:], z_t[:], t1[:], ALU.mult)
                        nc.vector.tensor_tensor(state[p][:], state[p][:], t1[:],
                                                ALU.add)

                    # F) field' = fe + state @ wo + bo
                    pf = psB.tile([128, N], F32, tag="psB", name="psbf")
                    for hf in range(2):
                        sl = slice(hf * 512, (hf + 1) * 512)
                        nc.tensor.matmul(pf[:, sl], wt["wobd"][:], state[p][:, sl],
                                         start=True, stop=False)
                        nc.tensor.matmul(pf[:, sl], wt["ieye"][:], fe_t[:, sl],
                                         start=False, stop=True)
                    nc.scalar.activation(field[p][:], pf[:], AF.Identity,
                                         bias=bs["bo"][:])

            # ---- decoder ----
            for p in range(PAIRS):
                dha = hab.tile([128, N], F32R, tag="hab", name="ha")
                dhb = hab.tile([128, N], F32R, tag="hab", name="hb")
                for (wname, bname, dst) in [("dw1A", "db1A", dha),
                                            ("dw1B", "db1B", dhb)]:
                    ph = psA.tile([128, N], F32, tag="psA", name="psah")
                    for hf in range(2):
                        sl = slice(hf * 512, (hf + 1) * 512)
                        nc.tensor.matmul(ph[:, sl], wt[wname][:], field[p][:, sl],
                                         start=True, stop=True)
                    nc.scalar.activation(dst[:], ph[:], AF.Relu, bias=bs[bname][:])
                po = psB.tile([2 * O, N], F32, tag="psB", name="psbo")
                for hf in range(2):
                    sl = slice(hf * 512, (hf + 1) * 512)
                    nc.tensor.matmul(po[:, sl], wt["dw2A"][:], dha[:, sl],
                                     start=True, stop=False)
                    nc.tensor.matmul(po[:, sl], wt["dw2B"][:], dhb[:, sl],
                                     start=False, stop=True)
                o2 = o2p.tile([2 * O, N], F32, tag="o2", name="o2")
                nc.scalar.activation(o2[:], po[:], AF.Identity, bias=bs["db2"][:])
                nc.sync.dma_start(out[2 * p, :, :], o2[0:O, :])
                nc.sync.dma_start(out[2 * p + 1, :, :], o2[O:2 * O, :])

    nc.compile()
    return nc


def _blockdiag(w):
    w = np.asarray(w, dtype=np.float32)
    r, c = w.shape
    out = np.zeros((2 * r, 2 * c), dtype=np.float32)
    out[:r, :c] = w
    out[r:, c:] = w
    return out


def _col1(v):
    return np.ascontiguousarray(np.asarray(v, dtype=np.float32).reshape(-1, 1))


def prepare(inputs):
    """Build host-packed per-core input maps and the compiled Bass module."""
    g = {k: np.asarray(v) for k, v in inputs.items()}
    pde_mix = float(np.asarray(g["pde_mix"], dtype=np.float64))
    alpha = float(1.0 / (1.0 + np.exp(-pde_mix)))
    dt_ = 1.0 / STEPS
    s2 = (1.0 - alpha) * dt_
    c0 = 1.0 - alpha * dt_

    enc_w1 = np.asarray(g["enc_w1"], np.float32)
    enc_w2 = np.asarray(g["enc_w2"], np.float32)
    pde_w1 = np.asarray(g["pde_w1"], np.float32)
    pde_w2 = np.asarray(g["pde_w2"], np.float32) * s2
    dec_w1 = np.asarray(g["dec_w1"], np.float32)
    dec_w2 = np.asarray(g["dec_w2"], np.float32)

    common = {
        "adj": np.ascontiguousarray(np.asarray(g["adj"], np.float32)),
        "w1eA": _blockdiag(enc_w1[:, 0:64]),
        "w1eB": _blockdiag(enc_w1[:, 64:128]),
        "w2eA": _blockdiag(enc_w2[0:64, :]),
        "w2eB": _blockdiag(enc_w2[64:128, :]),
        "pw1A": _blockdiag(pde_w1[:, 0:64]),
        "pw1B": _blockdiag(pde_w1[:, 64:128]),
        "pw2A": _blockdiag(pde_w2[0:64, :]),
        "pw2B": _blockdiag(pde_w2[64:128, :]),
        "wzbd": _blockdiag(np.asarray(g["ss_wz"], np.float32)),
        "uzbd": _blockdiag(np.asarray(g["ss_uz"], np.float32)),
        "whbd": _blockdiag(np.asarray(g["ss_wh"], np.float32)),
        "uhbd": _blockdiag(np.asarray(g["ss_uh"], np.float32)),
        "wobd": _blockdiag(np.asarray(g["ss_wo"], np.float32)),
        "dw1A": _blockdiag(dec_w1[:, 0:64]),
        "dw1B": _blockdiag(dec_w1[:, 64:128]),
        "dw2A": _blockdiag(dec_w2[0:64, :]),
        "dw2B": _blockdiag(dec_w2[64:128, :]),
        "ieye": np.eye(128, dtype=np.float32),
        "c0I": np.eye(128, dtype=np.float32) * np.float32(c0),
        "eb1A": _col1(np.tile(np.asarray(g["enc_b1"], np.float32)[0:64], 2)),
        "eb1B": _col1(np.tile(np.asarray(g["enc_b1"], np.float32)[64:128], 2)),
        "eb2": _col1(np.tile(np.asarray(g["enc_b2"], np.float32), 2)),
        "pb1A": _col1(np.tile(np.asarray(g["pde_b1"], np.float32)[0:64], 2)),
        "pb1B": _col1(np.tile(np.asarray(g["pde_b1"], np.float32)[64:128], 2)),
        "pb2": _col1(np.tile(np.asarray(g["pde_b2"], np.float32) * np.float32(s2), 2)),
        "bz": _col1(np.tile(np.asarray(g["ss_bz"], np.float32), 2)),
        "bh": _col1(np.tile(np.asarray(g["ss_bh"], np.float32), 2)),
        "bo": _col1(np.tile(np.asarray(g["ss_bo"], np.float32), 2)),
        "db1A": _col1(np.tile(np.asarray(g["dec_b1"], np.float32)[0:64], 2)),
        "db1B": _col1(np.tile(np.asarray(g["dec_b1"], np.float32)[64:128], 2)),
        "db2": _col1(np.tile(np.asarray(g["dec_b2"], np.float32), 2)),
    }

    hist = np.asarray(g["history_data"], np.float32)[..., 0]  # [B, L, N]
    in_maps = []
    for c in range(NCORES):
        m = dict(common)
        sl = hist[c * BL:(c + 1) * BL]                         # [BL, L, N]
        m["hist"] = np.ascontiguousarray(sl)
        in_maps.append(m)

    nc = _build(alpha)
    return nc, in_maps


def assemble(results):
    outs = [results[c]["out"] for c in range(NCORES)]          # [BL, O, N] each
    full = np.concatenate(outs, axis=0)                        # [B, O, N]
    return np.ascontiguousarray(full[..., None].astype(np.float32))


def kernel(**inputs) -> np.ndarray:
    nc, in_maps = prepare(inputs)
    res = run_bass_kernel_spmd(nc, in_maps, core_ids=list(range(NCORES)))
    return assemble(res.results)


# revision 5
# speedup vs baseline: 1.1043x; 1.1043x over previous
"""Trainium2 Bass kernel for nn_CLFMv2_NoTemporalEmb (graph-PDE message passing).

Strategy: data-parallel over batch B=64 across 8 NeuronCores (8 batches/core).
Per core, activations are stored "pair-packed feature-major":
    tensor[psi, tau],  psi = (batch_parity)*64 + d  (128 partitions),
                       tau = (batch_pair)*1024 + node  (1024 per pair tensor).
Every weight matmul uses block-diagonal [128,128] stationary operands so all
matmuls have K=128, M=128 and PSUM dst partition 0 (required for float32r).
The Laplacian A@field runs with PE-transposed field tiles as the stationary
operand against a precomputed, row-softmaxed, alpha*dt-scaled adjacency
transpose; the (1 - alpha*dt)*field residual is folded into its diagonal.
All matmuls use float32r (full PE rate, ~1e-4 relative rounding).
"""

import numpy as np

import concourse.bacc as bacc
import concourse.tile as tile
import concourse.mybir as mybir
from concourse.bass_utils import run_bass_kernel_spmd

F32 = mybir.dt.float32
F32R = mybir.dt.float32r
BF16 = mybir.dt.bfloat16
import os
MMDT = F32R if os.environ.get("KMM_DTYPE", "bf16") == "f32r" else BF16
AF = mybir.ActivationFunctionType
ALU = mybir.AluOpType

B, L, N, D, H, O = 64, 12, 1024, 64, 128, 12
STEPS = 4
NCORES = 8
BL = B // NCORES          # 8 batches per core
PAIRS = BL // 2           # 4
KCH = N // 128            # 8 adjacency chunks


def _build(alpha: float):
    dt_ = 1.0 / STEPS
    c_lap = alpha * dt_

    nc = bacc.Bacc("TRN2", target_bir_lowering=False, debug=False)

    def din(name, shape, dtype=MMDT):
        return nc.dram_tensor(name, shape, dtype, kind="ExternalInput")

    hist = din("hist", [BL, L, N])
    adj = din("adj", [N, N], F32)
    w1eA = din("w1eA", [2 * L, H])
    w1eB = din("w1eB", [2 * L, H])
    w2eA = din("w2eA", [H, 2 * D])
    w2eB = din("w2eB", [H, 2 * D])
    pw1A = din("pw1A", [2 * D, H])
    pw1B = din("pw1B", [2 * D, H])
    pw2A = din("pw2A", [H, 2 * D])
    pw2B = din("pw2B", [H, 2 * D])
    wzbd = din("wzbd", [2 * D, 2 * D])
    uzbd = din("uzbd", [2 * D, 2 * D])
    whbd = din("whbd", [2 * D, 2 * D])
    uhbd = din("uhbd", [2 * D, 2 * D])
    wobd = din("wobd", [2 * D, 2 * D])
    dw1A = din("dw1A", [2 * D, H])
    dw1B = din("dw1B", [2 * D, H])
    dw2A = din("dw2A", [H, 2 * O])
    dw2B = din("dw2B", [H, 2 * O])
    ieye = din("ieye", [128, 128])     # identity (transposes, +fe fold)
    c0I = din("c0I", [128, 128])       # -alpha*dt*I for adjacency diag

    bias_names = ["eb1A", "eb1B", "eb2", "pb1A", "pb1B", "pb2",
                  "bz", "bh", "bo", "db1A", "db1B"]
    biases = {n: din(n, [128, 1], F32) for n in bias_names}
    biases["db2"] = din("db2", [2 * O, 1], F32)

    out = nc.dram_tensor("out", [BL, O, N], F32, kind="ExternalOutput")

    with tile.TileContext(nc) as tc:
        import contextlib
        with contextlib.ExitStack() as ctx:
            pp = ctx.enter_context(tc.tile_pool(name="persist", bufs=1))
            hab = ctx.enter_context(tc.tile_pool(name="hab", bufs=4))
            ftp = ctx.enter_context(tc.tile_pool(name="ftp", bufs=2))
            tmp = ctx.enter_context(tc.tile_pool(name="tmp", bufs=2))
            ahp = ctx.enter_context(tc.tile_pool(name="ahp", bufs=2))
            adjp = ctx.enter_context(tc.tile_pool(name="adjp", bufs=2))
            smp = ctx.enter_context(tc.tile_pool(name="smp", bufs=4))
            fep = ctx.enter_context(tc.tile_pool(name="fep", bufs=2))
            zcp = ctx.enter_context(tc.tile_pool(name="zcp", bufs=4))
            x2p = ctx.enter_context(tc.tile_pool(name="x2p", bufs=2))
            o2p = ctx.enter_context(tc.tile_pool(name="o2p", bufs=2))
            psA = ctx.enter_context(tc.tile_pool(name="psA", bufs=2, space="PSUM"))
            psB = ctx.enter_context(tc.tile_pool(name="psB", bufs=2, space="PSUM"))

            # ---- load weights/biases into SBUF ----
            wt = {}
            for name, hnd in [("w1eA", w1eA), ("w1eB", w1eB)]:
                t = pp.tile([2 * L, H], MMDT, tag=name, name=name)
                nc.sync.dma_start(t[:], hnd[:, :])
                wt[name] = t
            for name, hnd in [("w2eA", w2eA), ("w2eB", w2eB),
                              ("pw1A", pw1A), ("pw1B", pw1B),
                              ("pw2A", pw2A), ("pw2B", pw2B),
                              ("wzbd", wzbd), ("uzbd", uzbd),
                              ("whbd", whbd), ("uhbd", uhbd),
                              ("wobd", wobd),
                              ("dw1A", dw1A), ("dw1B", dw1B),
                              ("ieye", ieye), ("c0I", c0I)]:
                t = pp.tile([128, 128], MMDT, tag=name, name=name)
                nc.sync.dma_start(t[:], hnd[:, :])
                wt[name] = t
            for name, hnd in [("dw2A", dw2A), ("dw2B", dw2B)]:
                t = pp.tile([H, 2 * O], MMDT, tag=name, name=name)
                nc.sync.dma_start(t[:], hnd[:, :])
                wt[name] = t
            bs = {}
            for name, hnd in biases.items():
                t = pp.tile([hnd.shape[0], 1], F32, tag="b_" + name, name="b_" + name)
                nc.sync.dma_start(t[:], hnd[:, :])
                bs[name] = t

            # ---- adjacency: row softmax (no max-sub; logits are tiny),
            #      scale by alpha*dt/rowsum, add (1-alpha*dt)I, transpose ----
            AT = pp.tile([128, KCH * N], MMDT, tag="AT", name="AT")  # [m', (k, n)]
            for c in range(KCH):
                ac = adjp.tile([128, N], F32, tag="adj", name="ac")
                nc.sync.dma_start(ac[:], adj[c * 128:(c + 1) * 128, :])
                rs = smp.tile([128, 1], F32, tag="small", name="rs")
                nc.scalar.activation(ac[:], ac[:], AF.Exp, accum_out=rs[:])
                rr = smp.tile([128, 1], F32, tag="small", name="rr")
                nc.vector.reciprocal(rr[:], rs[:])
                ah = ahp.tile([128, N], MMDT, tag="ah", name="ah")
                nc.vector.tensor_scalar(ah[:], ac[:], rr[:, 0:1], c_lap,
                                        ALU.mult, ALU.mult)
                # diagonal fold: rows c*128..c*128+127 own diag block k==c
                nc.vector.tensor_tensor(ah[:, c * 128:(c + 1) * 128],
                                        ah[:, c * 128:(c + 1) * 128],
                                        wt["c0I"][:], ALU.add)
                pt = psA.tile([128, N], MMDT, tag="psA", name="psat")
                for k in range(KCH):
                    nc.tensor.transpose(pt[:, k * 128:(k + 1) * 128],
                                        ah[:, k * 128:(k + 1) * 128],
                                        wt["ieye"][:])
                # strided evac: block (c,k) -> AT[:, k*1024 + c*128]
                nc.vector.tensor_copy(
                    AT[:].rearrange("p (k n) -> p k n", k=KCH)[:, :, c * 128:(c + 1) * 128],
                    pt[:].rearrange("p (k n) -> p k n", k=KCH),
                )

            # per-pair persistent activations
            field = [pp.tile([128, N], MMDT, tag=f"field{p}", name=f"field{p}") for p in range(PAIRS)]
            state = [pp.tile([128, N], MMDT, tag=f"state{p}", name=f"state{p}") for p in range(PAIRS)]

            # ---- encoder ----
            for p in range(PAIRS):
                # x pair-packed: xp[s*12+l, n] = hist[2p+s, l, n]
                xp = x2p.tile([2 * L, N], MMDT, tag="x2p", name="xp")
                nc.sync.dma_start(xp[0:L, :], hist[2 * p, :, :])
                nc.sync.dma_start(xp[L:2 * L, :], hist[2 * p + 1, :, :])
                hea = hab.tile([128, N], MMDT, tag="hab", name="hea")
                heb = hab.tile([128, N], MMDT, tag="hab", name="heb")
                for (wname, bname, dst) in [("w1eA", "eb1A", hea),
                                            ("w1eB", "eb1B", heb)]:
                    ph = psA.tile([128, N], F32, tag="psA", name="psah")
                    for hf in range(2):
                        sl = slice(hf * 512, (hf + 1) * 512)
                        nc.tensor.matmul(ph[:, sl], wt[wname][:], xp[:, sl],
                                         start=True, stop=True)
                    nc.scalar.activation(dst[:], ph[:], AF.Relu, bias=bs[bname][:])
                pf = psB.tile([128, N], F32, tag="psB", name="psbf")
                for hf in range(2):
                    sl = slice(hf * 512, (hf + 1) * 512)
                    nc.tensor.matmul(pf[:, sl], wt["w2eA"][:], hea[:, sl],
                                     start=True, stop=False)
                    nc.tensor.matmul(pf[:, sl], wt["w2eB"][:], heb[:, sl],
                                     start=False, stop=True)
                nc.scalar.activation(field[p][:], pf[:], AF.Identity,
                                     bias=bs["eb2"][:])

            # ---- main steps ----
            for s in range(STEPS):
                first = (s == 0)
                for p in range(PAIRS):
                    # A) transpose field pair -> fieldT [m', (k, psi)]
                    ptr = psA.tile([128, N], MMDT, tag="psA", name="psatr")
                    for k in range(KCH):
                        nc.tensor.transpose(ptr[:, k * 128:(k + 1) * 128],
                                            field[p][:, k * 128:(k + 1) * 128],
                                            wt["ieye"][:])
                    ft = ftp.tile([128, N], MMDT, tag="ft", name="ft")
                    nc.vector.tensor_copy(ft[:], ptr[:])

                    # B) pde layer 1: hA/hB = tanh(field @ w1 + b1)
                    ha = hab.tile([128, N], MMDT, tag="hab", name="ha")
                    hb = hab.tile([128, N], MMDT, tag="hab", name="hb")
                    for (wname, bname, dst) in [("pw1A", "pb1A", ha),
                                                ("pw1B", "pb1B", hb)]:
                        ph = psA.tile([128, N], F32, tag="psA", name="psah")
                        for hf in range(2):
                            sl = slice(hf * 512, (hf + 1) * 512)
                            nc.tensor.matmul(ph[:, sl], wt[wname][:],
                                             field[p][:, sl],
                                             start=True, stop=True)
                        nc.scalar.activation(dst[:], ph[:], AF.Tanh,
                                             bias=bs[bname][:])

                    # C) fe psum: Laplacian(+c0*field fold) + pde layer 2
                    fe_t = fep.tile([128, N], MMDT, tag="fe", name="fe_t")
                    z_t = zcp.tile([128, N], MMDT, tag="zc", name="z_t")
                    c_t = zcp.tile([128, N], MMDT, tag="zc", name="c_t")
                    pfe = psB.tile([128, N], F32, tag="psB", name="psbfe")
                    for hf in range(2):
                        sl = slice(hf * 512, (hf + 1) * 512)
                        for k in range(KCH):
                            nc.tensor.matmul(
                                pfe[:, sl],
                                ft[:, k * 128:(k + 1) * 128],
                                AT[:, k * N + hf * 512:k * N + (hf + 1) * 512],
                                start=(k == 0), stop=False)
                        nc.tensor.matmul(pfe[:, sl], wt["pw2A"][:], ha[:, sl],
                                         start=False, stop=False)
                        nc.tensor.matmul(pfe[:, sl], wt["pw2B"][:], hb[:, sl],
                                         start=False, stop=False)
                        nc.tensor.matmul(pfe[:, sl], wt["ieye"][:],
                                         field[p][:, sl],
                                         start=False, stop=True)
                    nc.scalar.activation(fe_t[:], pfe[:], AF.Identity,
                                         bias=bs["pb2"][:])

                    # D) GRU gates: z, cand
                    for (wname, uname, bname, func, dst) in [
                        ("wzbd", "uzbd", "bz", AF.Sigmoid, z_t),
                        ("whbd", "uhbd", "bh", AF.Tanh, c_t),
                    ]:
                        pz = psB.tile([128, N], F32, tag="psB", name="psbz")
                        for hf in range(2):
                            sl = slice(hf * 512, (hf + 1) * 512)
                            nc.tensor.matmul(pz[:, sl], wt[wname][:], fe_t[:, sl],
                                             start=True, stop=first)
                            if not first:
                                nc.tensor.matmul(pz[:, sl], wt[uname][:],
                                                 state[p][:, sl],
                                                 start=False, stop=True)
                        nc.scalar.activation(dst[:], pz[:], func, bias=bs[bname][:])

                    # E) state update
                    if first:
                        nc.vector.tensor_tensor(state[p][:], z_t[:], c_t[:],
                                                ALU.mult)
                    else:
                        t1 = tmp.tile([128, N], MMDT, tag="tmp", name="t1")
                        nc.vector.tensor_tensor(t1[:], c_t[:], state[p][:],
                                                ALU.subtract)
                        nc.vector.tensor_tensor(t1[:], z_t[:], t1[:], ALU.mult)
                        nc.vector.tensor_tensor(state[p][:], state[p][:], t1[:],
                                                ALU.add)

                    # F) field' = fe + state @ wo + bo
                    pf = psB.tile([128, N], F32, tag="psB", name="psbf")
                    for hf in range(2):
                        sl = slice(hf * 512, (hf + 1) * 512)
                        nc.tensor.matmul(pf[:, sl], wt["wobd"][:], state[p][:, sl],
                                         start=True, stop=False)
                        nc.tensor.matmul(pf[:, sl], wt["ieye"][:], fe_t[:, sl],
                                         start=False, stop=True)
                    nc.scalar.activation(field[p][:], pf[:], AF.Identity,
                                         bias=bs["bo"][:])

            # ---- decoder ----
            for p in range(PAIRS):
                dha = hab.tile([128, N], MMDT, tag="hab", name="ha")
                dhb = hab.tile([128, N], MMDT, tag="hab", name="hb")
                for (wname, bname, dst) in [("dw1A", "db1A", dha),
                                            ("dw1B", "db1B", dhb)]:
                    ph = psA.tile([128, N], F32, tag="psA", name="psah")
                    for hf in range(2):
                        sl = slice(hf * 512, (hf + 1) * 512)
                        nc.tensor.matmul(ph[:, sl], wt[wname][:], field[p][:, sl],
                                         start=True, stop=True)
                    nc.scalar.activation(dst[:], ph[:], AF.Relu, bias=bs[bname][:])
                po = psB.tile([2 * O, N], F32, tag="psB", name="psbo")
                for hf in range(2):
                    sl = slice(hf * 512, (hf + 1) * 512)
                    nc.tensor.matmul(po[:, sl], wt["dw2A"][:], dha[:, sl],
                                     start=True, stop=False)
                    nc.tensor.matmul(po[:, sl], wt["dw2B"][:], dhb[:, sl],
                                     start=False, stop=True)
                o2 = o2p.tile([2 * O, N], F32, tag="o2", name="o2")
                nc.scalar.activation(o2[:], po[:], AF.Identity, bias=bs["db2"][:])
                nc.sync.dma_start(out[2 * p, :, :], o2[0:O, :])
                nc.sync.dma_start(out[2 * p + 1, :, :], o2[O:2 * O, :])

    nc.compile()
    return nc


MMNP = mybir.dt.np(MMDT)


def _blockdiag(w):
    w = np.asarray(w, dtype=np.float32)
    r, c = w.shape
    out = np.zeros((2 * r, 2 * c), dtype=np.float32)
    out[:r, :c] = w
    out[r:, c:] = w
    return out.astype(MMNP)


def _col1(v):
    return np.ascontiguousarray(np.asarray(v, dtype=np.float32).reshape(-1, 1))


def prepare(inputs):
    """Build host-packed per-core input maps and the compiled Bass module."""
    g = {k: np.asarray(v) for k, v in inputs.items()}
    pde_mix = float(np.asarray(g["pde_mix"], dtype=np.float64))
    alpha = float(1.0 / (1.0 + np.exp(-pde_mix)))
    dt_ = 1.0 / STEPS
    s2 = (1.0 - alpha) * dt_

    enc_w1 = np.asarray(g["enc_w1"], np.float32)
    enc_w2 = np.asarray(g["enc_w2"], np.float32)
    pde_w1 = np.asarray(g["pde_w1"], np.float32)
    pde_w2 = np.asarray(g["pde_w2"], np.float32) * s2
    dec_w1 = np.asarray(g["dec_w1"], np.float32)
    dec_w2 = np.asarray(g["dec_w2"], np.float32)

    common = {
        "adj": np.ascontiguousarray(np.asarray(g["adj"], np.float32)),
        "w1eA": _blockdiag(enc_w1[:, 0:64]),
        "w1eB": _blockdiag(enc_w1[:, 64:128]),
        "w2eA": _blockdiag(enc_w2[0:64, :]),
        "w2eB": _blockdiag(enc_w2[64:128, :]),
        "pw1A": _blockdiag(pde_w1[:, 0:64]),
        "pw1B": _blockdiag(pde_w1[:, 64:128]),
        "pw2A": _blockdiag(pde_w2[0:64, :]),
        "pw2B": _blockdiag(pde_w2[64:128, :]),
        "wzbd": _blockdiag(np.asarray(g["ss_wz"], np.float32)),
        "uzbd": _blockdiag(np.asarray(g["ss_uz"], np.float32)),
        "whbd": _blockdiag(np.asarray(g["ss_wh"], np.float32)),
        "uhbd": _blockdiag(np.asarray(g["ss_uh"], np.float32)),
        "wobd": _blockdiag(np.asarray(g["ss_wo"], np.float32)),
        "dw1A": _blockdiag(dec_w1[:, 0:64]),
        "dw1B": _blockdiag(dec_w1[:, 64:128]),
        "dw2A": _blockdiag(dec_w2[0:64, :]),
        "dw2B": _blockdiag(dec_w2[64:128, :]),
        "ieye": np.eye(128, dtype=np.float32).astype(MMNP),
        "c0I": (np.eye(128, dtype=np.float32) * np.float32(-alpha * dt_)).astype(MMNP),
        "eb1A": _col1(np.tile(np.asarray(g["enc_b1"], np.float32)[0:64], 2)),
        "eb1B": _col1(np.tile(np.asarray(g["enc_b1"], np.float32)[64:128], 2)),
        "eb2": _col1(np.tile(np.asarray(g["enc_b2"], np.float32), 2)),
        "pb1A": _col1(np.tile(np.asarray(g["pde_b1"], np.float32)[0:64], 2)),
        "pb1B": _col1(np.tile(np.asarray(g["pde_b1"], np.float32)[64:128], 2)),
        "pb2": _col1(np.tile(np.asarray(g["pde_b2"], np.float32) * np.float32(s2), 2)),
        "bz": _col1(np.tile(np.asarray(g["ss_bz"], np.float32), 2)),
        "bh": _col1(np.tile(np.asarray(g["ss_bh"], np.float32), 2)),
        "bo": _col1(np.tile(np.asarray(g["ss_bo"], np.float32), 2)),
        "db1A": _col1(np.tile(np.asarray(g["dec_b1"], np.float32)[0:64], 2)),
        "db1B": _col1(np.tile(np.asarray(g["dec_b1"], np.float32)[64:128], 2)),
        "db2": _col1(np.tile(np.asarray(g["dec_b2"], np.float32), 2)),
    }

    hist = np.asarray(g["history_data"], np.float32)[..., 0]  # [B, L, N]
    in_maps = []
    for c in range(NCORES):
        m = dict(common)
        sl = hist[c * BL:(c + 1) * BL]                         # [BL, L, N]
        m["hist"] = np.ascontiguousarray(sl).astype(MMNP)
        in_maps.append(m)

    nc = _build(alpha)
    return nc, in_maps


def assemble(results):
    outs = [results[c]["out"] for c in range(NCORES)]          # [BL, O, N] each
    full = np.concatenate(outs, axis=0)                        # [B, O, N]
    return np.ascontiguousarray(full[..., None].astype(np.float32))


def kernel(**inputs) -> np.ndarray:
    nc, in_maps = prepare(inputs)
    res = run_bass_kernel_spmd(nc, in_maps, core_ids=list(range(NCORES)))
    return assemble(res.results)


# revision 7
# speedup vs baseline: 1.2710x; 1.1510x over previous
"""Trainium2 Bass kernel for nn_CLFMv2_NoTemporalEmb (graph-PDE message passing).

Strategy: data-parallel over batch B=64 across 8 NeuronCores (8 batches/core).
Per core, activations are stored "pair-packed feature-major":
    tensor[psi, tau],  psi = (batch_parity)*64 + d  (128 partitions),
                       tau = (batch_pair)*1024 + node  (1024 per pair tensor).
Every weight matmul uses block-diagonal [128,128] stationary operands so all
matmuls have K=128, M=128 and PSUM dst partition 0 (required for float32r).
The Laplacian A@field runs with PE-transposed field tiles as the stationary
operand against a precomputed, row-softmaxed, alpha*dt-scaled adjacency
transpose; the (1 - alpha*dt)*field residual is folded into its diagonal.
All matmuls use float32r (full PE rate, ~1e-4 relative rounding).
"""

import numpy as np

import concourse.bacc as bacc
import concourse.tile as tile
import concourse.mybir as mybir
from concourse.bass_utils import run_bass_kernel_spmd

F32 = mybir.dt.float32
F32R = mybir.dt.float32r
BF16 = mybir.dt.bfloat16
import os
MMDT = F32R if os.environ.get("KMM_DTYPE", "bf16") == "f32r" else BF16
AF = mybir.ActivationFunctionType
ALU = mybir.AluOpType

B, L, N, D, H, O = 64, 12, 1024, 64, 128, 12
STEPS = 4
NCORES = 8
BL = B // NCORES          # 8 batches per core
PAIRS = BL // 2           # 4
KCH = N // 128            # 8 adjacency chunks


def _build(alpha: float):
    dt_ = 1.0 / STEPS
    c_lap = alpha * dt_

    nc = bacc.Bacc("TRN2", target_bir_lowering=False, debug=False)

    def din(name, shape, dtype=MMDT):
        return nc.dram_tensor(name, shape, dtype, kind="ExternalInput")

    hist = din("hist", [BL, L, N])
    adj = din("adj", [N, N], F32)
    w1eA = din("w1eA", [2 * L, H])
    w1eB = din("w1eB", [2 * L, H])
    w2eA = din("w2eA", [H, 2 * D])
    w2eB = din("w2eB", [H, 2 * D])
    pw1A = din("pw1A", [2 * D, H])
    pw1B = din("pw1B", [2 * D, H])
    pw2A = din("pw2A", [H, 2 * D])
    pw2B = din("pw2B", [H, 2 * D])
    wzbd = din("wzbd", [2 * D, 2 * D])
    uzbd = din("uzbd", [2 * D, 2 * D])
    whbd = din("whbd", [2 * D, 2 * D])
    uhbd = din("uhbd", [2 * D, 2 * D])
    wobd = din("wobd", [2 * D, 2 * D])
    dw1A = din("dw1A", [2 * D, H])
    dw1B = din("dw1B", [2 * D, H])
    dw2A = din("dw2A", [H, 2 * O])
    dw2B = din("dw2B", [H, 2 * O])
    ieye = din("ieye", [128, 128])     # identity (transposes, +fe fold)
    c0I = din("c0I", [128, 128])       # -alpha*dt*I for adjacency diag

    bias_names = ["eb1A", "eb1B", "eb2", "pb1A", "pb1B", "pb2",
                  "bz", "bh", "bo", "db1A", "db1B"]
    biases = {n: din(n, [128, 1], F32) for n in bias_names}
    biases["db2"] = din("db2", [2 * O, 1], F32)

    out = nc.dram_tensor("out", [BL, O, N], F32, kind="ExternalOutput")

    with tile.TileContext(nc) as tc:
        import contextlib
        with contextlib.ExitStack() as ctx:
            pp = ctx.enter_context(tc.tile_pool(name="persist", bufs=1))
            hab = ctx.enter_context(tc.tile_pool(name="hab", bufs=10))
            ftp = ctx.enter_context(tc.tile_pool(name="ftp", bufs=5))
            tmp = ctx.enter_context(tc.tile_pool(name="tmp", bufs=2))
            ahp = ctx.enter_context(tc.tile_pool(name="ahp", bufs=2))
            adjp = ctx.enter_context(tc.tile_pool(name="adjp", bufs=2))
            smp = ctx.enter_context(tc.tile_pool(name="smp", bufs=4))
            fep = ctx.enter_context(tc.tile_pool(name="fep", bufs=5))
            zcp = ctx.enter_context(tc.tile_pool(name="zcp", bufs=4))
            x2p = ctx.enter_context(tc.tile_pool(name="x2p", bufs=2))
            o2p = ctx.enter_context(tc.tile_pool(name="o2p", bufs=2))
            psA = ctx.enter_context(tc.tile_pool(name="psA", bufs=2, space="PSUM"))
            psB = ctx.enter_context(tc.tile_pool(name="psB", bufs=2, space="PSUM"))

            # ---- load weights/biases into SBUF ----
            wt = {}
            for name, hnd in [("w1eA", w1eA), ("w1eB", w1eB)]:
                t = pp.tile([2 * L, H], MMDT, tag=name, name=name)
                nc.sync.dma_start(t[:], hnd[:, :])
                wt[name] = t
            for name, hnd in [("w2eA", w2eA), ("w2eB", w2eB),
                              ("pw1A", pw1A), ("pw1B", pw1B),
                              ("pw2A", pw2A), ("pw2B", pw2B),
                              ("wzbd", wzbd), ("uzbd", uzbd),
                              ("whbd", whbd), ("uhbd", uhbd),
                              ("wobd", wobd),
                              ("dw1A", dw1A), ("dw1B", dw1B),
                              ("ieye", ieye), ("c0I", c0I)]:
                t = pp.tile([128, 128], MMDT, tag=name, name=name)
                nc.sync.dma_start(t[:], hnd[:, :])
                wt[name] = t
            for name, hnd in [("dw2A", dw2A), ("dw2B", dw2B)]:
                t = pp.tile([H, 2 * O], MMDT, tag=name, name=name)
                nc.sync.dma_start(t[:], hnd[:, :])
                wt[name] = t
            bs = {}
            for name, hnd in biases.items():
                t = pp.tile([hnd.shape[0], 1], F32, tag="b_" + name, name="b_" + name)
                nc.sync.dma_start(t[:], hnd[:, :])
                bs[name] = t

            # ---- adjacency: row softmax (no max-sub; logits are tiny),
            #      scale by alpha*dt/rowsum, add (1-alpha*dt)I, transpose ----
            AT = pp.tile([128, KCH * N], MMDT, tag="AT", name="AT")  # [m', (k, n)]
            for c in range(KCH):
                ac = adjp.tile([128, N], F32, tag="adj", name="ac")
                nc.sync.dma_start(ac[:], adj[c * 128:(c + 1) * 128, :])
                rs = smp.tile([128, 1], F32, tag="small", name="rs")
                nc.scalar.activation(ac[:], ac[:], AF.Exp, accum_out=rs[:])
                rr = smp.tile([128, 1], F32, tag="small", name="rr")
                nc.vector.reciprocal(rr[:], rs[:])
                ah = ahp.tile([128, N], MMDT, tag="ah", name="ah")
                nc.vector.tensor_scalar(ah[:], ac[:], rr[:, 0:1], c_lap,
                                        ALU.mult, ALU.mult)
                # diagonal fold: rows c*128..c*128+127 own diag block k==c
                nc.vector.tensor_tensor(ah[:, c * 128:(c + 1) * 128],
                                        ah[:, c * 128:(c + 1) * 128],
                                        wt["c0I"][:], ALU.add)
                pt = psA.tile([128, N], F32, tag="psA", name="psat")
                for k in range(KCH):
                    nc.tensor.matmul(pt[:, k * 128:(k + 1) * 128],
                                     ah[:, k * 128:(k + 1) * 128],
                                     wt["ieye"][:], start=True, stop=True)
                # strided evac: block (c,k) -> AT[:, k*1024 + c*128]
                nc.vector.tensor_copy(
                    AT[:].rearrange("p (k n) -> p k n", k=KCH)[:, :, c * 128:(c + 1) * 128],
                    pt[:].rearrange("p (k n) -> p k n", k=KCH),
                )

            # per-pair persistent activations
            field = [pp.tile([128, N], MMDT, tag=f"field{p}", name=f"field{p}") for p in range(PAIRS)]
            state = [pp.tile([128, N], MMDT, tag=f"state{p}", name=f"state{p}") for p in range(PAIRS)]

            # ---- encoder ----
            for p in range(PAIRS):
                # x pair-packed: xp[s*12+l, n] = hist[2p+s, l, n]
                xp = x2p.tile([2 * L, N], MMDT, tag="x2p", name="xp")
                nc.sync.dma_start(xp[0:L, :], hist[2 * p, :, :])
                nc.sync.dma_start(xp[L:2 * L, :], hist[2 * p + 1, :, :])
                hea = hab.tile([128, N], MMDT, tag="hab", name="hea")
                heb = hab.tile([128, N], MMDT, tag="hab", name="heb")
                for (wname, bname, dst) in [("w1eA", "eb1A", hea),
                                            ("w1eB", "eb1B", heb)]:
                    ph = psA.tile([128, N], F32, tag="psA", name="psah")
                    for hf in range(2):
                        sl = slice(hf * 512, (hf + 1) * 512)
                        nc.tensor.matmul(ph[:, sl], wt[wname][:], xp[:, sl],
                                         start=True, stop=True)
                    nc.scalar.activation(dst[:], ph[:], AF.Relu, bias=bs[bname][:])
                pf = psB.tile([128, N], F32, tag="psB", name="psbf")
                for hf in range(2):
                    sl = slice(hf * 512, (hf + 1) * 512)
                    nc.tensor.matmul(pf[:, sl], wt["w2eA"][:], hea[:, sl],
                                     start=True, stop=False)
                    nc.tensor.matmul(pf[:, sl], wt["w2eB"][:], heb[:, sl],
                                     start=False, stop=True)
                nc.scalar.activation(field[p][:], pf[:], AF.Identity,
                                     bias=bs["eb2"][:])

            # ---- main steps (phase-major for PE pipelining) ----
            for s in range(STEPS):
                first = (s == 0)
                fts, has, hbs = [], [], []
                for p in range(PAIRS):
                    # A) transpose field pair -> fieldT [m', (k, psi)]
                    ptr = psA.tile([128, N], F32, tag="psA", name="psatr")
                    for k in range(KCH):
                        nc.tensor.matmul(ptr[:, k * 128:(k + 1) * 128],
                                         field[p][:, k * 128:(k + 1) * 128],
                                         wt["ieye"][:], start=True, stop=True)
                    ft = ftp.tile([128, N], MMDT, tag="ft", name="ft")
                    nc.vector.tensor_copy(ft[:], ptr[:])
                    fts.append(ft)

                    # B) pde layer 1: hA/hB = tanh(field @ w1 + b1)
                    ha = hab.tile([128, N], MMDT, tag="hab", name="ha")
                    hb = hab.tile([128, N], MMDT, tag="hab", name="hb")
                    for (wname, bname, dst) in [("pw1A", "pb1A", ha),
                                                ("pw1B", "pb1B", hb)]:
                        ph = psA.tile([128, N], F32, tag="psA", name="psah")
                        for hf in range(2):
                            sl = slice(hf * 512, (hf + 1) * 512)
                            nc.tensor.matmul(ph[:, sl], wt[wname][:],
                                             field[p][:, sl],
                                             start=True, stop=True)
                        nc.scalar.activation(dst[:], ph[:], AF.Tanh,
                                             bias=bs[bname][:])
                    has.append(ha)
                    hbs.append(hb)

                fes, sts = [], []
                for p in range(PAIRS):
                    ft, ha, hb = fts[p], has[p], hbs[p]
                    # C) fe psum: Laplacian(-c_lap diag) + pde layer 2 + field
                    fe_t = fep.tile([128, N], MMDT, tag="fe", name="fe_t")
                    z_t = zcp.tile([128, N], MMDT, tag="zc", name="z_t")
                    c_t = zcp.tile([128, N], MMDT, tag="zc", name="c_t")
                    pfe = psB.tile([128, N], F32, tag="psB", name="psbfe")
                    for hf in range(2):
                        sl = slice(hf * 512, (hf + 1) * 512)
                        for k in range(KCH):
                            nc.tensor.matmul(
                                pfe[:, sl],
                                ft[:, k * 128:(k + 1) * 128],
                                AT[:, k * N + hf * 512:k * N + (hf + 1) * 512],
                                start=(k == 0), stop=False)
                        nc.tensor.matmul(pfe[:, sl], wt["pw2A"][:], ha[:, sl],
                                         start=False, stop=False)
                        nc.tensor.matmul(pfe[:, sl], wt["pw2B"][:], hb[:, sl],
                                         start=False, stop=False)
                        nc.tensor.matmul(pfe[:, sl], wt["ieye"][:],
                                         field[p][:, sl],
                                         start=False, stop=True)
                    nc.scalar.activation(fe_t[:], pfe[:], AF.Identity,
                                         bias=bs["pb2"][:])

                    # D) GRU gates: z, cand
                    for (wname, uname, bname, func, dst) in [
                        ("wzbd", "uzbd", "bz", AF.Sigmoid, z_t),
                        ("whbd", "uhbd", "bh", AF.Tanh, c_t),
                    ]:
                        pz = psB.tile([128, N], F32, tag="psB", name="psbz")
                        for hf in range(2):
                            sl = slice(hf * 512, (hf + 1) * 512)
                            nc.tensor.matmul(pz[:, sl], wt[wname][:], fe_t[:, sl],
                                             start=True, stop=first)
                            if not first:
                                nc.tensor.matmul(pz[:, sl], wt[uname][:],
                                                 state[p][:, sl],
                                                 start=False, stop=True)
                        nc.scalar.activation(dst[:], pz[:], func, bias=bs[bname][:])

                    # E) state update
                    if first:
                        nc.vector.tensor_tensor(state[p][:], z_t[:], c_t[:],
                                                ALU.mult)
                    else:
                        t1 = tmp.tile([128, N], MMDT, tag="tmp", name="t1")
                        nc.vector.tensor_tensor(t1[:], c_t[:], state[p][:],
                                                ALU.subtract)
                        nc.vector.tensor_tensor(t1[:], z_t[:], t1[:], ALU.mult)
                        nc.vector.tensor_tensor(state[p][:], state[p][:], t1[:],
                                                ALU.add)
                    fes.append(fe_t)

                for p in range(PAIRS):
                    # F) field' = fe + state @ wo + bo
                    pf = psB.tile([128, N], F32, tag="psB", name="psbf")
                    for hf in range(2):
                        sl = slice(hf * 512, (hf + 1) * 512)
                        nc.tensor.matmul(pf[:, sl], wt["wobd"][:], state[p][:, sl],
                                         start=True, stop=False)
                        nc.tensor.matmul(pf[:, sl], wt["ieye"][:], fes[p][:, sl],
                                         start=False, stop=True)
                    nc.scalar.activation(field[p][:], pf[:], AF.Identity,
                                         bias=bs["bo"][:])

            # ---- decoder ----
            for p in range(PAIRS):
                dha = hab.tile([128, N], MMDT, tag="hab", name="ha")
                dhb = hab.tile([128, N], MMDT, tag="hab", name="hb")
                for (wname, bname, dst) in [("dw1A", "db1A", dha),
                                            ("dw1B", "db1B", dhb)]:
                    ph = psA.tile([128, N], F32, tag="psA", name="psah")
                    for hf in range(2):
                        sl = slice(hf * 512, (hf + 1) * 512)
                        nc.tensor.matmul(ph[:, sl], wt[wname][:], field[p][:, sl],
                                         start=True, stop=True)
                    nc.scalar.activation(dst[:], ph[:], AF.Relu, bias=bs[bname][:])
                po = psB.tile([2 * O, N], F32, tag="psB", name="psbo")
                for hf in range(2):
                    sl = slice(hf * 512, (hf + 1) * 512)
                    nc.tensor.matmul(po[:, sl], wt["dw2A"][:], dha[:, sl],
                                     start=True, stop=False)
                    nc.tensor.matmul(po[:, sl], wt["dw2B"][:], dhb[:, sl],
                                     start=False, stop=True)
                o2 = o2p.tile([2 * O, N], F32, tag="o2", name="o2")
                nc.scalar.activation(o2[:], po[:], AF.Identity, bias=bs["db2"][:])
                nc.sync.dma_start(out[2 * p, :, :], o2[0:O, :])
                nc.sync.dma_start(out[2 * p + 1, :, :], o2[O:2 * O, :])

    nc.compile()
    return nc


MMNP = mybir.dt.np(MMDT)


def _blockdiag(w):
    w = np.asarray(w, dtype=np.float32)
    r, c = w.shape
    out = np.zeros((2 * r, 2 * c), dtype=np.float32)
    out[:r, :c] = w
    out[r:, c:] = w
    return out.astype(MMNP)


def _col1(v):
    return np.ascontiguousarray(np.asarray(v, dtype=np.float32).reshape(-1, 1))


def prepare(inputs):
    """Build host-packed per-core input maps and the compiled Bass module."""
    g = {k: np.asarray(v) for k, v in inputs.items()}
    pde_mix = float(np.asarray(g["pde_mix"], dtype=np.float64))
    alpha = float(1.0 / (1.0 + np.exp(-pde_mix)))
    dt_ = 1.0 / STEPS
    s2 = (1.0 - alpha) * dt_

    enc_w1 = np.asarray(g["enc_w1"], np.float32)
    enc_w2 = np.asarray(g["enc_w2"], np.float32)
    pde_w1 = np.asarray(g["pde_w1"], np.float32)
    pde_w2 = np.asarray(g["pde_w2"], np.float32) * s2
    dec_w1 = np.asarray(g["dec_w1"], np.float32)
    dec_w2 = np.asarray(g["dec_w2"], np.float32)

    common = {
        "adj": np.ascontiguousarray(np.asarray(g["adj"], np.float32)),
        "w1eA": _blockdiag(enc_w1[:, 0:64]),
        "w1eB": _blockdiag(enc_w1[:, 64:128]),
        "w2eA": _blockdiag(enc_w2[0:64, :]),
        "w2eB": _blockdiag(enc_w2[64:128, :]),
        "pw1A": _blockdiag(pde_w1[:, 0:64]),
        "pw1B": _blockdiag(pde_w1[:, 64:128]),
        "pw2A": _blockdiag(pde_w2[0:64, :]),
        "pw2B": _blockdiag(pde_w2[64:128, :]),
        "wzbd": _blockdiag(np.asarray(g["ss_wz"], np.float32)),
        "uzbd": _blockdiag(np.asarray(g["ss_uz"], np.float32)),
        "whbd": _blockdiag(np.asarray(g["ss_wh"], np.float32)),
        "uhbd": _blockdiag(np.asarray(g["ss_uh"], np.float32)),
        "wobd": _blockdiag(np.asarray(g["ss_wo"], np.float32)),
        "dw1A": _blockdiag(dec_w1[:, 0:64]),
        "dw1B": _blockdiag(dec_w1[:, 64:128]),
        "dw2A": _blockdiag(dec_w2[0:64, :]),
        "dw2B": _blockdiag(dec_w2[64:128, :]),
        "ieye": np.eye(128, dtype=np.float32).astype(MMNP),
        "c0I": (np.eye(128, dtype=np.float32) * np.float32(-alpha * dt_)).astype(MMNP),
        "eb1A": _col1(np.tile(np.asarray(g["enc_b1"], np.float32)[0:64], 2)),
        "eb1B": _col1(np.tile(np.asarray(g["enc_b1"], np.float32)[64:128], 2)),
        "eb2": _col1(np.tile(np.asarray(g["enc_b2"], np.float32), 2)),
        "pb1A": _col1(np.tile(np.asarray(g["pde_b1"], np.float32)[0:64], 2)),
        "pb1B": _col1(np.tile(np.asarray(g["pde_b1"], np.float32)[64:128], 2)),
        "pb2": _col1(np.tile(np.asarray(g["pde_b2"], np.float32) * np.float32(s2), 2)),
        "bz": _col1(np.tile(np.asarray(g["ss_bz"], np.float32), 2)),
        "bh": _col1(np.tile(np.asarray(g["ss_bh"], np.float32), 2)),
        "bo": _col1(np.tile(np.asarray(g["ss_bo"], np.float32), 2)),
        "db1A": _col1(np.tile(np.asarray(g["dec_b1"], np.float32)[0:64], 2)),
        "db1B": _col1(np.tile(np.asarray(g["dec_b1"], np.float32)[64:128], 2)),
        "db2": _col1(np.tile(np.asarray(g["dec_b2"], np.float32), 2)),
    }

    hist = np.asarray(g["history_data"], np.float32)[..., 0]  # [B, L, N]
    in_maps = []
    for c in range(NCORES):
        m = dict(common)
        sl = hist[c * BL:(c + 1) * BL]                         # [BL, L, N]
        m["hist"] = np.ascontiguousarray(sl).astype(MMNP)
        in_maps.append(m)

    nc = _build(alpha)
    return nc, in_maps


def assemble(results):
    outs = [results[c]["out"] for c in range(NCORES)]          # [BL, O, N] each
    full = np.concatenate(outs, axis=0)                        # [B, O, N]
    return np.ascontiguousarray(full[..., None].astype(np.float32))


def kernel(**inputs) -> np.ndarray:
    nc, in_maps = prepare(inputs)
    res = run_bass_kernel_spmd(nc, in_maps, core_ids=list(range(NCORES)))
    return assemble(res.results)


# revision 8
# speedup vs baseline: 1.4462x; 1.1378x over previous
"""Trainium2 Bass kernel for nn_CLFMv2_NoTemporalEmb (graph-PDE message passing).

Strategy: data-parallel over batch B=64 across 8 NeuronCores (8 batches/core).
Per core, activations are stored "pair-packed feature-major":
    tensor[psi, tau],  psi = (batch_parity)*64 + d  (128 partitions),
                       tau = (batch_pair)*1024 + node  (1024 per pair tensor).
Every weight matmul uses block-diagonal [128,128] stationary operands so all
matmuls have K=128, M=128 and PSUM dst partition 0 (required for float32r).
The Laplacian A@field runs with PE-transposed field tiles as the stationary
operand against a precomputed, row-softmaxed, alpha*dt-scaled adjacency
transpose; the (1 - alpha*dt)*field residual is folded into its diagonal.
All matmuls use float32r (full PE rate, ~1e-4 relative rounding).
"""

import numpy as np

import concourse.bacc as bacc
import concourse.tile as tile
import concourse.mybir as mybir
from concourse.bass_utils import run_bass_kernel_spmd

F32 = mybir.dt.float32
F32R = mybir.dt.float32r
BF16 = mybir.dt.bfloat16
import os
MMDT = F32R if os.environ.get("KMM_DTYPE", "bf16") == "f32r" else BF16
AF = mybir.ActivationFunctionType
ALU = mybir.AluOpType

B, L, N, D, H, O = 64, 12, 1024, 64, 128, 12
STEPS = 4
NCORES = 8
BL = B // NCORES          # 8 batches per core
PAIRS = BL // 2           # 4
KCH = N // 128            # 8 adjacency chunks


def _build(alpha: float):
    dt_ = 1.0 / STEPS
    c_lap = alpha * dt_

    nc = bacc.Bacc("TRN2", target_bir_lowering=False, debug=False)

    def din(name, shape, dtype=MMDT):
        return nc.dram_tensor(name, shape, dtype, kind="ExternalInput")

    hist = din("hist", [BL, L, N])
    ath = din("ath", [128, KCH * N])   # host-preprocessed c_lap*(A - I), transposed
    w1eA = din("w1eA", [2 * L, H])
    w1eB = din("w1eB", [2 * L, H])
    w2eA = din("w2eA", [H, 2 * D])
    w2eB = din("w2eB", [H, 2 * D])
    pw1A = din("pw1A", [2 * D, H])
    pw1B = din("pw1B", [2 * D, H])
    pw2A = din("pw2A", [H, 2 * D])
    pw2B = din("pw2B", [H, 2 * D])
    wzbd = din("wzbd", [2 * D, 2 * D])
    uzbd = din("uzbd", [2 * D, 2 * D])
    whbd = din("whbd", [2 * D, 2 * D])
    uhbd = din("uhbd", [2 * D, 2 * D])
    wobd = din("wobd", [2 * D, 2 * D])
    dw1A = din("dw1A", [2 * D, H])
    dw1B = din("dw1B", [2 * D, H])
    dw2A = din("dw2A", [H, 2 * O])
    dw2B = din("dw2B", [H, 2 * O])
    ieye = din("ieye", [128, 128])     # identity (transposes, +fe fold)

    bias_names = ["eb1A", "eb1B", "eb2", "pb1A", "pb1B",
                  "bz", "bh", "bo", "db1A", "db1B"]
    biases = {n: din(n, [128, 1], F32) for n in bias_names}
    biases["db2"] = din("db2", [2 * O, 1], F32)

    out = nc.dram_tensor("out", [BL, O, N], F32, kind="ExternalOutput")

    with tile.TileContext(nc) as tc:
        import contextlib
        with contextlib.ExitStack() as ctx:
            pp = ctx.enter_context(tc.tile_pool(name="persist", bufs=1))
            hab = ctx.enter_context(tc.tile_pool(name="hab", bufs=10))
            ftp = ctx.enter_context(tc.tile_pool(name="ftp", bufs=5))
            tmp = ctx.enter_context(tc.tile_pool(name="tmp", bufs=2))
            fep = ctx.enter_context(tc.tile_pool(name="fep", bufs=5))
            zcp = ctx.enter_context(tc.tile_pool(name="zcp", bufs=4))
            x2p = ctx.enter_context(tc.tile_pool(name="x2p", bufs=2))
            o2p = ctx.enter_context(tc.tile_pool(name="o2p", bufs=2))
            psA = ctx.enter_context(tc.tile_pool(name="psA", bufs=2, space="PSUM"))
            psB = ctx.enter_context(tc.tile_pool(name="psB", bufs=2, space="PSUM"))

            # ---- load weights/biases into SBUF ----
            wt = {}
            for name, hnd in [("w1eA", w1eA), ("w1eB", w1eB)]:
                t = pp.tile([2 * L, H], MMDT, tag=name, name=name)
                nc.sync.dma_start(t[:], hnd[:, :])
                wt[name] = t
            for name, hnd in [("w2eA", w2eA), ("w2eB", w2eB),
                              ("pw1A", pw1A), ("pw1B", pw1B),
                              ("pw2A", pw2A), ("pw2B", pw2B),
                              ("wzbd", wzbd), ("uzbd", uzbd),
                              ("whbd", whbd), ("uhbd", uhbd),
                              ("wobd", wobd),
                              ("dw1A", dw1A), ("dw1B", dw1B),
                              ("ieye", ieye)]:
                t = pp.tile([128, 128], MMDT, tag=name, name=name)
                nc.sync.dma_start(t[:], hnd[:, :])
                wt[name] = t
            for name, hnd in [("dw2A", dw2A), ("dw2B", dw2B)]:
                t = pp.tile([H, 2 * O], MMDT, tag=name, name=name)
                nc.sync.dma_start(t[:], hnd[:, :])
                wt[name] = t
            bs = {}
            for name, hnd in biases.items():
                t = pp.tile([hnd.shape[0], 1], F32, tag="b_" + name, name="b_" + name)
                nc.sync.dma_start(t[:], hnd[:, :])
                bs[name] = t

            # ---- adjacency operator: host-precomputed, one DMA ----
            AT = pp.tile([128, KCH * N], MMDT, tag="AT", name="AT")
            nc.sync.dma_start(AT[:], ath[:, :])

            # per-pair persistent activations
            field = [pp.tile([128, N], MMDT, tag=f"field{p}", name=f"field{p}") for p in range(PAIRS)]
            state = [pp.tile([128, N], MMDT, tag=f"state{p}", name=f"state{p}") for p in range(PAIRS)]

            # ---- encoder ----
            for p in range(PAIRS):
                # x pair-packed: xp[s*12+l, n] = hist[2p+s, l, n]
                xp = x2p.tile([2 * L, N], MMDT, tag="x2p", name="xp")
                nc.sync.dma_start(xp[0:L, :], hist[2 * p, :, :])
                nc.sync.dma_start(xp[L:2 * L, :], hist[2 * p + 1, :, :])
                hea = hab.tile([128, N], MMDT, tag="hab", name="hea")
                heb = hab.tile([128, N], MMDT, tag="hab", name="heb")
                for (wname, bname, dst) in [("w1eA", "eb1A", hea),
                                            ("w1eB", "eb1B", heb)]:
                    ph = psA.tile([128, N], F32, tag="psA", name="psah")
                    for hf in range(2):
                        sl = slice(hf * 512, (hf + 1) * 512)
                        nc.tensor.matmul(ph[:, sl], wt[wname][:], xp[:, sl],
                                         start=True, stop=True)
                    nc.scalar.activation(dst[:], ph[:], AF.Relu, bias=bs[bname][:])
                pf = psB.tile([128, N], F32, tag="psB", name="psbf")
                for hf in range(2):
                    sl = slice(hf * 512, (hf + 1) * 512)
                    nc.tensor.matmul(pf[:, sl], wt["w2eA"][:], hea[:, sl],
                                     start=True, stop=False)
                    nc.tensor.matmul(pf[:, sl], wt["w2eB"][:], heb[:, sl],
                                     start=False, stop=True)
                nc.scalar.activation(field[p][:], pf[:], AF.Identity,
                                     bias=bs["eb2"][:])

            # ---- main steps (phase-major for PE pipelining) ----
            for s in range(STEPS):
                first = (s == 0)
                fts, has, hbs = [], [], []
                for p in range(PAIRS):
                    # A) transpose field pair -> fieldT [m', (k, psi)]
                    ptr = psA.tile([128, N], F32, tag="psA", name="psatr")
                    for k in range(KCH):
                        nc.tensor.matmul(ptr[:, k * 128:(k + 1) * 128],
                                         field[p][:, k * 128:(k + 1) * 128],
                                         wt["ieye"][:], start=True, stop=True)
                    ft = ftp.tile([128, N], MMDT, tag="ft", name="ft")
                    nc.vector.tensor_copy(ft[:], ptr[:])
                    fts.append(ft)

                    # B) pde layer 1: hA/hB = tanh(field @ w1 + b1)
                    ha = hab.tile([128, N], MMDT, tag="hab", name="ha")
                    hb = hab.tile([128, N], MMDT, tag="hab", name="hb")
                    for (wname, bname, dst) in [("pw1A", "pb1A", ha),
                                                ("pw1B", "pb1B", hb)]:
                        ph = psA.tile([128, N], F32, tag="psA", name="psah")
                        for hf in range(2):
                            sl = slice(hf * 512, (hf + 1) * 512)
                            nc.tensor.matmul(ph[:, sl], wt[wname][:],
                                             field[p][:, sl],
                                             start=True, stop=True)
                        nc.scalar.activation(dst[:], ph[:], AF.Tanh,
                                             bias=bs[bname][:])
                    has.append(ha)
                    hbs.append(hb)

                fes, sts = [], []
                for p in range(PAIRS):
                    ft, ha, hb = fts[p], has[p], hbs[p]
                    # C) fe psum: Laplacian(-c_lap diag) + pde layer 2 + field
                    fe_t = fep.tile([128, N], MMDT, tag="fe", name="fe_t")
                    z_t = zcp.tile([128, N], MMDT, tag="zc", name="z_t")
                    c_t = zcp.tile([128, N], MMDT, tag="zc", name="c_t")
                    pfe = psB.tile([128, N], F32, tag="psB", name="psbfe")
                    for hf in range(2):
                        sl = slice(hf * 512, (hf + 1) * 512)
                        for k in range(KCH):
                            nc.tensor.matmul(
                                pfe[:, sl],
                                ft[:, k * 128:(k + 1) * 128],
                                AT[:, k * N + hf * 512:k * N + (hf + 1) * 512],
                                start=(k == 0), stop=False)
                        nc.tensor.matmul(pfe[:, sl], wt["pw2A"][:], ha[:, sl],
                                         start=False, stop=False)
                        nc.tensor.matmul(pfe[:, sl], wt["pw2B"][:], hb[:, sl],
                                         start=False, stop=True)
                    nc.vector.tensor_tensor(fe_t[:], pfe[:], field[p][:],
                                            ALU.add)

                    # D) GRU gates: z, cand
                    for (wname, uname, bname, func, dst) in [
                        ("wzbd", "uzbd", "bz", AF.Sigmoid, z_t),
                        ("whbd", "uhbd", "bh", AF.Tanh, c_t),
                    ]:
                        pz = psB.tile([128, N], F32, tag="psB", name="psbz")
                        for hf in range(2):
                            sl = slice(hf * 512, (hf + 1) * 512)
                            nc.tensor.matmul(pz[:, sl], wt[wname][:], fe_t[:, sl],
                                             start=True, stop=first)
                            if not first:
                                nc.tensor.matmul(pz[:, sl], wt[uname][:],
                                                 state[p][:, sl],
                                                 start=False, stop=True)
                        nc.scalar.activation(dst[:], pz[:], func, bias=bs[bname][:])

                    # E) state update
                    if first:
                        nc.vector.tensor_tensor(state[p][:], z_t[:], c_t[:],
                                                ALU.mult)
                    else:
                        t1 = tmp.tile([128, N], MMDT, tag="tmp", name="t1")
                        nc.vector.tensor_tensor(t1[:], c_t[:], state[p][:],
                                                ALU.subtract)
                        nc.vector.tensor_tensor(t1[:], z_t[:], t1[:], ALU.mult)
                        nc.vector.tensor_tensor(state[p][:], state[p][:], t1[:],
                                                ALU.add)
                    fes.append(fe_t)

                for p in range(PAIRS):
                    # F) field' = fe + state @ wo + bo
                    pf = psB.tile([128, N], F32, tag="psB", name="psbf")
                    for hf in range(2):
                        sl = slice(hf * 512, (hf + 1) * 512)
                        nc.tensor.matmul(pf[:, sl], wt["wobd"][:], state[p][:, sl],
                                         start=True, stop=False)
                        nc.tensor.matmul(pf[:, sl], wt["ieye"][:], fes[p][:, sl],
                                         start=False, stop=True)
                    nc.scalar.activation(field[p][:], pf[:], AF.Identity,
                                         bias=bs["bo"][:])

            # ---- decoder ----
            for p in range(PAIRS):
                dha = hab.tile([128, N], MMDT, tag="hab", name="ha")
                dhb = hab.tile([128, N], MMDT, tag="hab", name="hb")
                for (wname, bname, dst) in [("dw1A", "db1A", dha),
                                            ("dw1B", "db1B", dhb)]:
                    ph = psA.tile([128, N], F32, tag="psA", name="psah")
                    for hf in range(2):
                        sl = slice(hf * 512, (hf + 1) * 512)
                        nc.tensor.matmul(ph[:, sl], wt[wname][:], field[p][:, sl],
                                         start=True, stop=True)
                    nc.scalar.activation(dst[:], ph[:], AF.Relu, bias=bs[bname][:])
                po = psB.tile([2 * O, N], F32, tag="psB", name="psbo")
                for hf in range(2):
                    sl = slice(hf * 512, (hf + 1) * 512)
                    nc.tensor.matmul(po[:, sl], wt["dw2A"][:], dha[:, sl],
                                     start=True, stop=False)
                    nc.tensor.matmul(po[:, sl], wt["dw2B"][:], dhb[:, sl],
                                     start=False, stop=True)
                o2 = o2p.tile([2 * O, N], F32, tag="o2", name="o2")
                nc.scalar.activation(o2[:], po[:], AF.Identity, bias=bs["db2"][:])
                nc.sync.dma_start(out[2 * p, :, :], o2[0:O, :])
                nc.sync.dma_start(out[2 * p + 1, :, :], o2[O:2 * O, :])

    nc.compile()
    return nc


MMNP = mybir.dt.np(MMDT)


def _blockdiag(w):
    w = np.asarray(w, dtype=np.float32)
    r, c = w.shape
    out = np.zeros((2 * r, 2 * c), dtype=np.float32)
    out[:r, :c] = w
    out[r:, c:] = w
    return out.astype(MMNP)


def _col1(v):
    return np.ascontiguousarray(np.asarray(v, dtype=np.float32).reshape(-1, 1))


def prepare(inputs):
    """Build host-packed per-core input maps and the compiled Bass module."""
    g = {k: np.asarray(v) for k, v in inputs.items()}
    pde_mix = float(np.asarray(g["pde_mix"], dtype=np.float64))
    alpha = float(1.0 / (1.0 + np.exp(-pde_mix)))
    dt_ = 1.0 / STEPS
    s2 = (1.0 - alpha) * dt_

    enc_w1 = np.asarray(g["enc_w1"], np.float32)
    enc_w2 = np.asarray(g["enc_w2"], np.float32)
    pde_w1 = np.asarray(g["pde_w1"], np.float32)
    pde_w2 = np.asarray(g["pde_w2"], np.float32) * s2
    dec_w1 = np.asarray(g["dec_w1"], np.float32)
    dec_w2 = np.asarray(g["dec_w2"], np.float32)

    adj64 = np.asarray(g["adj"], np.float64)
    e = np.exp(adj64 - adj64.max(axis=-1, keepdims=True))
    A = e / e.sum(axis=-1, keepdims=True)
    c_lap = alpha * dt_
    M = c_lap * (A - np.eye(N))
    ath = M.T.reshape(KCH, 128, N).transpose(1, 0, 2).reshape(128, KCH * N)

    pb2d = (np.asarray(g["pde_b2"], np.float64) * s2)
    bz_f = np.asarray(g["ss_bz"], np.float64) + pb2d @ np.asarray(g["ss_wz"], np.float64)
    bh_f = np.asarray(g["ss_bh"], np.float64) + pb2d @ np.asarray(g["ss_wh"], np.float64)
    bo_f = np.asarray(g["ss_bo"], np.float64) + pb2d

    common = {
        "ath": np.ascontiguousarray(ath.astype(np.float32)).astype(MMNP),
        "w1eA": _blockdiag(enc_w1[:, 0:64]),
        "w1eB": _blockdiag(enc_w1[:, 64:128]),
        "w2eA": _blockdiag(enc_w2[0:64, :]),
        "w2eB": _blockdiag(enc_w2[64:128, :]),
        "pw1A": _blockdiag(pde_w1[:, 0:64]),
        "pw1B": _blockdiag(pde_w1[:, 64:128]),
        "pw2A": _blockdiag(pde_w2[0:64, :]),
        "pw2B": _blockdiag(pde_w2[64:128, :]),
        "wzbd": _blockdiag(np.asarray(g["ss_wz"], np.float32)),
        "uzbd": _blockdiag(np.asarray(g["ss_uz"], np.float32)),
        "whbd": _blockdiag(np.asarray(g["ss_wh"], np.float32)),
        "uhbd": _blockdiag(np.asarray(g["ss_uh"], np.float32)),
        "wobd": _blockdiag(np.asarray(g["ss_wo"], np.float32)),
        "dw1A": _blockdiag(dec_w1[:, 0:64]),
        "dw1B": _blockdiag(dec_w1[:, 64:128]),
        "dw2A": _blockdiag(dec_w2[0:64, :]),
        "dw2B": _blockdiag(dec_w2[64:128, :]),
        "ieye": np.eye(128, dtype=np.float32).astype(MMNP),
        "eb1A": _col1(np.tile(np.asarray(g["enc_b1"], np.float32)[0:64], 2)),
        "eb1B": _col1(np.tile(np.asarray(g["enc_b1"], np.float32)[64:128], 2)),
        "eb2": _col1(np.tile(np.asarray(g["enc_b2"], np.float32), 2)),
        "pb1A": _col1(np.tile(np.asarray(g["pde_b1"], np.float32)[0:64], 2)),
        "pb1B": _col1(np.tile(np.asarray(g["pde_b1"], np.float32)[64:128], 2)),
        "bz": _col1(np.tile(bz_f.astype(np.float32), 2)),
        "bh": _col1(np.tile(bh_f.astype(np.float32), 2)),
        "bo": _col1(np.tile(bo_f.astype(np.float32), 2)),
        "db1A": _col1(np.tile(np.asarray(g["dec_b1"], np.float32)[0:64], 2)),
        "db1B": _col1(np.tile(np.asarray(g["dec_b1"], np.float32)[64:128], 2)),
        "db2": _col1(np.tile(np.asarray(g["dec_b2"], np.float32), 2)),
    }

    hist = np.asarray(g["history_data"], np.float32)[..., 0]  # [B, L, N]
    in_maps = []
    for c in range(NCORES):
        m = dict(common)
        sl = hist[c * BL:(c + 1) * BL]                         # [BL, L, N]
        m["hist"] = np.ascontiguousarray(sl).astype(MMNP)
        in_maps.append(m)

    nc = _build(alpha)
    return nc, in_maps


def assemble(results):
    outs = [results[c]["out"] for c in range(NCORES)]          # [BL, O, N] each
    full = np.concatenate(outs, axis=0)                        # [B, O, N]
    return np.ascontiguousarray(full[..., None].astype(np.float32))


def kernel(**inputs) -> np.ndarray:
    nc, in_maps = prepare(inputs)
    res = run_bass_kernel_spmd(nc, in_maps, core_ids=list(range(NCORES)))
    return assemble(res.results)


# revision 11
# speedup vs baseline: 1.5177x; 1.0494x over previous
"""Trainium2 Bass kernel for nn_CLFMv2_NoTemporalEmb (graph-PDE message passing).

Strategy: data-parallel over batch B=64 across 8 NeuronCores (8 batches/core).
Per core, activations are stored "pair-packed feature-major":
    tensor[psi, tau],  psi = (batch_parity)*64 + d  (128 partitions),
                       tau = (batch_pair)*1024 + node  (1024 per pair tensor).
Every weight matmul uses block-diagonal [128,128] stationary operands so all
matmuls have K=128, M=128 and PSUM dst partition 0 (required for float32r).
The Laplacian A@field runs with PE-transposed field tiles as the stationary
operand against a precomputed, row-softmaxed, alpha*dt-scaled adjacency
transpose; the (1 - alpha*dt)*field residual is folded into its diagonal.
All matmuls use float32r (full PE rate, ~1e-4 relative rounding).
"""

import numpy as np

import concourse.bacc as bacc
import concourse.tile as tile
import concourse.mybir as mybir
from concourse.bass_utils import run_bass_kernel_spmd

F32 = mybir.dt.float32
F32R = mybir.dt.float32r
BF16 = mybir.dt.bfloat16
import os
MMDT = F32R if os.environ.get("KMM_DTYPE", "bf16") == "f32r" else BF16
AF = mybir.ActivationFunctionType
ALU = mybir.AluOpType

B, L, N, D, H, O = 64, 12, 1024, 64, 128, 12
STEPS = 4
NCORES = 8
BL = B // NCORES          # 8 batches per core
PAIRS = BL // 2           # 4
KCH = N // 128            # 8 adjacency chunks


def _build(alpha: float):
    dt_ = 1.0 / STEPS
    c_lap = alpha * dt_

    nc = bacc.Bacc("TRN2", target_bir_lowering=False, debug=False)

    def din(name, shape, dtype=MMDT):
        return nc.dram_tensor(name, shape, dtype, kind="ExternalInput")

    hist = din("hist", [BL, L, N])
    ath = din("ath", [128, KCH * N])   # host-preprocessed c_lap*(A - I), transposed
    w1eA = din("w1eA", [2 * L, H])
    w1eB = din("w1eB", [2 * L, H])
    w2eA = din("w2eA", [H, 2 * D])
    w2eB = din("w2eB", [H, 2 * D])
    pw1A = din("pw1A", [2 * D, H])
    pw1B = din("pw1B", [2 * D, H])
    pw2A = din("pw2A", [H, 2 * D])
    pw2B = din("pw2B", [H, 2 * D])
    wzbd = din("wzbd", [2 * D, 2 * D])
    uzbd = din("uzbd", [2 * D, 2 * D])
    whbd = din("whbd", [2 * D, 2 * D])
    uhbd = din("uhbd", [2 * D, 2 * D])
    wobd = din("wobd", [2 * D, 2 * D])
    dw1A = din("dw1A", [2 * D, H])
    dw1B = din("dw1B", [2 * D, H])
    dw2A = din("dw2A", [H, 2 * O])
    dw2B = din("dw2B", [H, 2 * O])
    ieye = din("ieye", [128, 128])     # identity (transposes, +fe fold)

    bias_names = ["eb1A", "eb1B", "eb2", "pb1A", "pb1B",
                  "bz", "bh", "bo", "db1A", "db1B"]
    biases = {n: din(n, [128, 1], F32) for n in bias_names}
    biases["db2"] = din("db2", [2 * O, 1], F32)

    out = nc.dram_tensor("out", [BL, O, N], F32, kind="ExternalOutput")

    with tile.TileContext(nc) as tc:
        import contextlib
        with contextlib.ExitStack() as ctx:
            pp = ctx.enter_context(tc.tile_pool(name="persist", bufs=1))
            hab = ctx.enter_context(tc.tile_pool(name="hab", bufs=10))
            ftp = ctx.enter_context(tc.tile_pool(name="ftp", bufs=5))
            tmp = ctx.enter_context(tc.tile_pool(name="tmp", bufs=2))
            fep = ctx.enter_context(tc.tile_pool(name="fep", bufs=5))
            zcp = ctx.enter_context(tc.tile_pool(name="zcp", bufs=4))
            x2p = ctx.enter_context(tc.tile_pool(name="x2p", bufs=2))
            o2p = ctx.enter_context(tc.tile_pool(name="o2p", bufs=2))
            psA = ctx.enter_context(tc.tile_pool(name="psA", bufs=2, space="PSUM"))
            psB = ctx.enter_context(tc.tile_pool(name="psB", bufs=2, space="PSUM"))

            # ---- load weights/biases into SBUF ----
            wt = {}
            for name, hnd in [("w1eA", w1eA), ("w1eB", w1eB)]:
                t = pp.tile([2 * L, H], MMDT, tag=name, name=name)
                nc.sync.dma_start(t[:], hnd[:, :])
                wt[name] = t
            for name, hnd in [("w2eA", w2eA), ("w2eB", w2eB),
                              ("pw1A", pw1A), ("pw1B", pw1B),
                              ("pw2A", pw2A), ("pw2B", pw2B),
                              ("wzbd", wzbd), ("uzbd", uzbd),
                              ("whbd", whbd), ("uhbd", uhbd),
                              ("wobd", wobd),
                              ("dw1A", dw1A), ("dw1B", dw1B),
                              ("ieye", ieye)]:
                t = pp.tile([128, 128], MMDT, tag=name, name=name)
                nc.sync.dma_start(t[:], hnd[:, :])
                wt[name] = t
            for name, hnd in [("dw2A", dw2A), ("dw2B", dw2B)]:
                t = pp.tile([H, 2 * O], MMDT, tag=name, name=name)
                nc.sync.dma_start(t[:], hnd[:, :])
                wt[name] = t
            bs = {}
            for name, hnd in biases.items():
                t = pp.tile([hnd.shape[0], 1], F32, tag="b_" + name, name="b_" + name)
                nc.sync.dma_start(t[:], hnd[:, :])
                bs[name] = t

            # ---- adjacency operator: host-precomputed, one DMA ----
            AT = pp.tile([128, KCH * N], MMDT, tag="AT", name="AT")
            nc.sync.dma_start(AT[:], ath[:, :])

            # per-pair persistent activations
            field = [pp.tile([128, N], MMDT, tag=f"field{p}", name=f"field{p}") for p in range(PAIRS)]
            state = [pp.tile([128, N], MMDT, tag=f"state{p}", name=f"state{p}") for p in range(PAIRS)]

            E, Ob = slice(0, 64), slice(64, 128)

            def mm_split(ps_t, sl, wname, rhs_ap_e, rhs_ap_o, start, stop,
                         wslice_e=(E, E), wslice_o=(Ob, Ob)):
                """col-tiled even/odd half matmuls (K=64, M=64 each)."""
                w = wt[wname]
                nc.tensor.matmul(ps_t[E, sl], w[wslice_e[0], wslice_e[1]],
                                 rhs_ap_e, start=start, stop=stop,
                                 tile_position=(0, 0))
                nc.tensor.matmul(ps_t[Ob, sl], w[wslice_o[0], wslice_o[1]],
                                 rhs_ap_o, start=start, stop=stop,
                                 tile_position=(64, 64), skip_group_check=True)

            # ---- encoder ----
            for p in range(PAIRS):
                # x pair-packed: xp[s*12+l, n] = hist[2p+s, l, n]
                xp = x2p.tile([2 * L, N], MMDT, tag="x2p", name="xp")
                nc.sync.dma_start(xp[0:L, :], hist[2 * p, :, :])
                nc.sync.dma_start(xp[L:2 * L, :], hist[2 * p + 1, :, :])
                hea = hab.tile([128, N], MMDT, tag="hab", name="hea")
                heb = hab.tile([128, N], MMDT, tag="hab", name="heb")
                for (wname, bname, dst) in [("w1eA", "eb1A", hea),
                                            ("w1eB", "eb1B", heb)]:
                    ph = psA.tile([128, N], F32, tag="psA", name="psah")
                    for hf in range(2):
                        sl = slice(hf * 512, (hf + 1) * 512)
                        nc.tensor.matmul(ph[:, sl], wt[wname][:], xp[:, sl],
                                         start=True, stop=True)
                    nc.scalar.activation(dst[:], ph[:], AF.Relu, bias=bs[bname][:])
                pf = psB.tile([128, N], F32, tag="psB", name="psbf")
                for hf in range(2):
                    sl = slice(hf * 512, (hf + 1) * 512)
                    mm_split(pf, sl, "w2eA", hea[E, sl], hea[Ob, sl],
                             True, False)
                    mm_split(pf, sl, "w2eB", heb[E, sl], heb[Ob, sl],
                             False, True)
                nc.scalar.activation(field[p][:], pf[:], AF.Identity,
                                     bias=bs["eb2"][:])

            # ---- main steps (phase-major for PE pipelining) ----
            for s in range(STEPS):
                first = (s == 0)
                fts, has, hbs = [], [], []
                for p in range(PAIRS):
                    # A) transpose field pair -> fieldT [m', (k, psi)]
                    ptr = psA.tile([128, N], F32, tag="psA", name="psatr")
                    for k in range(KCH):
                        nc.tensor.matmul(ptr[:, k * 128:(k + 1) * 128],
                                         field[p][:, k * 128:(k + 1) * 128],
                                         wt["ieye"][:], start=True, stop=True)
                    ft = ftp.tile([128, N], MMDT, tag="ft", name="ft")
                    nc.vector.tensor_copy(ft[:], ptr[:])
                    fts.append(ft)

                    # B) pde layer 1: hA/hB = tanh(field @ w1 + b1)
                    ha = hab.tile([128, N], MMDT, tag="hab", name="ha")
                    hb = hab.tile([128, N], MMDT, tag="hab", name="hb")
                    for (wname, bname, dst) in [("pw1A", "pb1A", ha),
                                                ("pw1B", "pb1B", hb)]:
                        ph = psA.tile([128, N], F32, tag="psA", name="psah")
                        for hf in range(2):
                            sl = slice(hf * 512, (hf + 1) * 512)
                            mm_split(ph, sl, wname, field[p][E, sl],
                                     field[p][Ob, sl], True, True)
                        nc.scalar.activation(dst[:], ph[:], AF.Tanh,
                                             bias=bs[bname][:])
                    has.append(ha)
                    hbs.append(hb)

                fes = []
                zcs = []

                def emit_gru(p, fe_t):
                    z_t = zcp.tile([128, N], MMDT, tag="zc", name="z_t")
                    c_t = zcp.tile([128, N], MMDT, tag="zc", name="c_t")
                    for (wname, uname, bname, func, dst) in [
                        ("wzbd", "uzbd", "bz", AF.Sigmoid, z_t),
                        ("whbd", "uhbd", "bh", AF.Tanh, c_t),
                    ]:
                        pz = psB.tile([128, N], F32, tag="psB", name="psbz")
                        for hf in range(2):
                            sl = slice(hf * 512, (hf + 1) * 512)
                            mm_split(pz, sl, wname, fe_t[E, sl], fe_t[Ob, sl],
                                     True, first)
                            if not first:
                                mm_split(pz, sl, uname, state[p][E, sl],
                                         state[p][Ob, sl], False, True)
                        nc.scalar.activation(dst[:], pz[:], func, bias=bs[bname][:])
                    # state update
                    if first:
                        nc.vector.tensor_tensor(state[p][:], z_t[:], c_t[:],
                                                ALU.mult)
                    else:
                        t1 = tmp.tile([128, N], MMDT, tag="tmp", name="t1")
                        nc.vector.tensor_tensor(t1[:], c_t[:], state[p][:],
                                                ALU.subtract)
                        nc.vector.tensor_tensor(t1[:], z_t[:], t1[:], ALU.mult)
                        nc.vector.tensor_tensor(state[p][:], state[p][:], t1[:],
                                                ALU.add)

                for p in range(PAIRS):
                    ft, ha, hb = fts[p], has[p], hbs[p]
                    # C) fe psum: Laplacian(-c_lap diag) + pde layer 2 + field
                    fe_t = fep.tile([128, N], MMDT, tag="fe", name="fe_t")
                    pfe = psB.tile([128, N], F32, tag="psB", name="psbfe")
                    for hf in range(2):
                        sl = slice(hf * 512, (hf + 1) * 512)
                        for k in range(KCH):
                            nc.tensor.matmul(
                                pfe[:, sl],
                                ft[:, k * 128:(k + 1) * 128],
                                AT[:, k * N + hf * 512:k * N + (hf + 1) * 512],
                                start=(k == 0), stop=False)
                        nc.tensor.matmul(pfe[:, sl], wt["pw2A"][:], ha[:, sl],
                                         start=False, stop=False)
                        nc.tensor.matmul(pfe[:, sl], wt["pw2B"][:], hb[:, sl],
                                         start=False, stop=True)
                    nc.vector.tensor_tensor(fe_t[:], pfe[:], field[p][:],
                                            ALU.add)
                    fes.append(fe_t)
                    # D+E) GRU for the PREVIOUS pair overlaps this Laplacian
                    if p > 0:
                        emit_gru(p - 1, fes[p - 1])
                emit_gru(PAIRS - 1, fes[PAIRS - 1])

                for p in range(PAIRS):
                    # F) field' = fe + state @ wo + bo
                    pf = psB.tile([128, N], F32, tag="psB", name="psbf")
                    for hf in range(2):
                        sl = slice(hf * 512, (hf + 1) * 512)
                        mm_split(pf, sl, "wobd", state[p][E, sl],
                                 state[p][Ob, sl], True, False)
                        mm_split(pf, sl, "ieye", fes[p][E, sl],
                                 fes[p][Ob, sl], False, True)
                    nc.vector.tensor_scalar(field[p][:], pf[:], bs["bo"][:, 0:1],
                                            None, ALU.add)

            # ---- decoder ----
            for p in range(PAIRS):
                dha = hab.tile([128, N], MMDT, tag="hab", name="ha")
                dhb = hab.tile([128, N], MMDT, tag="hab", name="hb")
                for (wname, bname, dst) in [("dw1A", "db1A", dha),
                                            ("dw1B", "db1B", dhb)]:
                    ph = psA.tile([128, N], F32, tag="psA", name="psah")
                    for hf in range(2):
                        sl = slice(hf * 512, (hf + 1) * 512)
                        mm_split(ph, sl, wname, field[p][E, sl],
                                 field[p][Ob, sl], True, True)
                    nc.scalar.activation(dst[:], ph[:], AF.Relu, bias=bs[bname][:])
                po = psB.tile([2 * O, N], F32, tag="psB", name="psbo")
                for hf in range(2):
                    sl = slice(hf * 512, (hf + 1) * 512)
                    nc.tensor.matmul(po[:, sl], wt["dw2A"][:], dha[:, sl],
                                     start=True, stop=False)
                    nc.tensor.matmul(po[:, sl], wt["dw2B"][:], dhb[:, sl],
                                     start=False, stop=True)
                o2 = o2p.tile([2 * O, N], F32, tag="o2", name="o2")
                nc.scalar.activation(o2[:], po[:], AF.Identity, bias=bs["db2"][:])
                nc.sync.dma_start(out[2 * p, :, :], o2[0:O, :])
                nc.sync.dma_start(out[2 * p + 1, :, :], o2[O:2 * O, :])

    nc.compile()
    return nc


MMNP = mybir.dt.np(MMDT)


def _blockdiag(w):
    w = np.asarray(w, dtype=np.float32)
    r, c = w.shape
    out = np.zeros((2 * r, 2 * c), dtype=np.float32)
    out[:r, :c] = w
    out[r:, c:] = w
    return out.astype(MMNP)


def _col1(v):
    return np.ascontiguousarray(np.asarray(v, dtype=np.float32).reshape(-1, 1))


def prepare(inputs):
    """Build host-packed per-core input maps and the compiled Bass module."""
    g = {k: np.asarray(v) for k, v in inputs.items()}
    pde_mix = float(np.asarray(g["pde_mix"], dtype=np.float64))
    alpha = float(1.0 / (1.0 + np.exp(-pde_mix)))
    dt_ = 1.0 / STEPS
    s2 = (1.0 - alpha) * dt_

    enc_w1 = np.asarray(g["enc_w1"], np.float32)
    enc_w2 = np.asarray(g["enc_w2"], np.float32)
    pde_w1 = np.asarray(g["pde_w1"], np.float32)
    pde_w2 = np.asarray(g["pde_w2"], np.float32) * s2
    dec_w1 = np.asarray(g["dec_w1"], np.float32)
    dec_w2 = np.asarray(g["dec_w2"], np.float32)

    adj64 = np.asarray(g["adj"], np.float64)
    e = np.exp(adj64 - adj64.max(axis=-1, keepdims=True))
    A = e / e.sum(axis=-1, keepdims=True)
    c_lap = alpha * dt_
    M = c_lap * (A - np.eye(N))
    ath = M.T.reshape(KCH, 128, N).transpose(1, 0, 2).reshape(128, KCH * N)

    pb2d = (np.asarray(g["pde_b2"], np.float64) * s2)
    bz_f = np.asarray(g["ss_bz"], np.float64) + pb2d @ np.asarray(g["ss_wz"], np.float64)
    bh_f = np.asarray(g["ss_bh"], np.float64) + pb2d @ np.asarray(g["ss_wh"], np.float64)
    bo_f = np.asarray(g["ss_bo"], np.float64) + pb2d

    common = {
        "ath": np.ascontiguousarray(ath.astype(np.float32)).astype(MMNP),
        "w1eA": _blockdiag(enc_w1[:, 0:64]),
        "w1eB": _blockdiag(enc_w1[:, 64:128]),
        "w2eA": _blockdiag(enc_w2[0:64, :]),
        "w2eB": _blockdiag(enc_w2[64:128, :]),
        "pw1A": _blockdiag(pde_w1[:, 0:64]),
        "pw1B": _blockdiag(pde_w1[:, 64:128]),
        "pw2A": _blockdiag(pde_w2[0:64, :]),
        "pw2B": _blockdiag(pde_w2[64:128, :]),
        "wzbd": _blockdiag(np.asarray(g["ss_wz"], np.float32)),
        "uzbd": _blockdiag(np.asarray(g["ss_uz"], np.float32)),
        "whbd": _blockdiag(np.asarray(g["ss_wh"], np.float32)),
        "uhbd": _blockdiag(np.asarray(g["ss_uh"], np.float32)),
        "wobd": _blockdiag(np.asarray(g["ss_wo"], np.float32)),
        "dw1A": _blockdiag(dec_w1[:, 0:64]),
        "dw1B": _blockdiag(dec_w1[:, 64:128]),
        "dw2A": _blockdiag(dec_w2[0:64, :]),
        "dw2B": _blockdiag(dec_w2[64:128, :]),
        "ieye": np.eye(128, dtype=np.float32).astype(MMNP),
        "eb1A": _col1(np.tile(np.asarray(g["enc_b1"], np.float32)[0:64], 2)),
        "eb1B": _col1(np.tile(np.asarray(g["enc_b1"], np.float32)[64:128], 2)),
        "eb2": _col1(np.tile(np.asarray(g["enc_b2"], np.float32), 2)),
        "pb1A": _col1(np.tile(np.asarray(g["pde_b1"], np.float32)[0:64], 2)),
        "pb1B": _col1(np.tile(np.asarray(g["pde_b1"], np.float32)[64:128], 2)),
        "bz": _col1(np.tile(bz_f.astype(np.float32), 2)),
        "bh": _col1(np.tile(bh_f.astype(np.float32), 2)),
        "bo": _col1(np.tile(bo_f.astype(np.float32), 2)),
        "db1A": _col1(np.tile(np.asarray(g["dec_b1"], np.float32)[0:64], 2)),
        "db1B": _col1(np.tile(np.asarray(g["dec_b1"], np.float32)[64:128], 2)),
        "db2": _col1(np.tile(np.asarray(g["dec_b2"], np.float32), 2)),
    }

    hist = np.asarray(g["history_data"], np.float32)[..., 0]  # [B, L, N]
    in_maps = []
    for c in range(NCORES):
        m = dict(common)
        sl = hist[c * BL:(c + 1) * BL]                         # [BL, L, N]
        m["hist"] = np.ascontiguousarray(sl).astype(MMNP)
        in_maps.append(m)

    nc = _build(alpha)
    return nc, in_maps


def assemble(results):
    outs = [results[c]["out"] for c in range(NCORES)]          # [BL, O, N] each
    full = np.concatenate(outs, axis=0)                        # [B, O, N]
    return np.ascontiguousarray(full[..., None].astype(np.float32))


def kernel(**inputs) -> np.ndarray:
    nc, in_maps = prepare(inputs)
    res = run_bass_kernel_spmd(nc, in_maps, core_ids=list(range(NCORES)))
    return assemble(res.results)


# revision 12
# speedup vs baseline: 1.7861x; 1.1769x over previous
"""Trainium2 Bass kernel for nn_CLFMv2_NoTemporalEmb (graph-PDE message passing).

Strategy: data-parallel over batch B=64 across 8 NeuronCores (8 batches/core).
Per core, activations are "pair-packed feature-major":
    tensor[psi, n],  psi = (batch_parity)*64 + d  (128 partitions),
    one [128, 1024] tensor per batch-pair (4 pairs/core).
Weight matmuls use block-diagonal [128,128] stationary operands so K=128,
M=128, PSUM dst partition 0. The Laplacian A@field uses PE-transposed field
tiles (regular matmuls against identity so HAM stays warm) as stationary
operands against the host-precomputed alpha*dt*(softmax(adj) - I) transpose;
the softmax and all weight packing run on host in float64.
Matmuls run in bf16 (full PE rate); PSUM accumulates fp32.
"""

import os
import contextlib

import numpy as np

import concourse.bacc as bacc
import concourse.tile as tile
import concourse.mybir as mybir
from concourse.bass_utils import run_bass_kernel_spmd

F32 = mybir.dt.float32
F32R = mybir.dt.float32r
BF16 = mybir.dt.bfloat16
MMDT = F32R if os.environ.get("KMM_DTYPE", "bf16") == "f32r" else BF16
AF = mybir.ActivationFunctionType
ALU = mybir.AluOpType

B, L, N, D, H, O = 64, 12, 1024, 64, 128, 12
STEPS = 4
NCORES = 8
BL = B // NCORES          # 8 batches per core
PAIRS = BL // 2           # 4
KCH = N // 128            # 8 adjacency chunks

# weight-pack slot order (each slot is a [128, 128] block in wpk)
WNAMES = ["w1eA", "w1eB", "w2eA", "w2eB", "pw1A", "pw1B", "pw2A", "pw2B",
          "wzbd", "uzbd", "whbd", "uhbd", "wobd", "dw1A", "dw1B",
          "dw2A", "dw2B", "ieye"]
BNAMES = ["eb1A", "eb1B", "eb2", "pb1A", "pb1B", "bz", "bh", "bo",
          "db1A", "db1B", "db2"]


def _build():
    nc = bacc.Bacc("TRN2", target_bir_lowering=False, debug=False)

    wpk = nc.dram_tensor("wpk", [128, len(WNAMES) * 128], MMDT,
                         kind="ExternalInput")
    bpk = nc.dram_tensor("bpk", [128, len(BNAMES)], F32, kind="ExternalInput")
    hist = nc.dram_tensor("hist", [BL, L, N], MMDT, kind="ExternalInput")
    ath = nc.dram_tensor("ath", [128, KCH * N], MMDT, kind="ExternalInput")
    out = nc.dram_tensor("out", [BL, O, N], F32, kind="ExternalOutput")

    with tile.TileContext(nc) as tc:
        with contextlib.ExitStack() as ctx:
            pp = ctx.enter_context(tc.tile_pool(name="persist", bufs=1))
            hab = ctx.enter_context(tc.tile_pool(name="hab", bufs=10))
            ftp = ctx.enter_context(tc.tile_pool(name="ftp", bufs=5))
            tmp = ctx.enter_context(tc.tile_pool(name="tmp", bufs=2))
            fep = ctx.enter_context(tc.tile_pool(name="fep", bufs=5))
            zcp = ctx.enter_context(tc.tile_pool(name="zcp", bufs=4))
            x2p = ctx.enter_context(tc.tile_pool(name="x2p", bufs=2))
            o2p = ctx.enter_context(tc.tile_pool(name="o2p", bufs=2))
            psA = ctx.enter_context(tc.tile_pool(name="psA", bufs=2, space="PSUM"))
            psB = ctx.enter_context(tc.tile_pool(name="psB", bufs=2, space="PSUM"))

            # ---- packed weights and biases: two DMAs ----
            wpkt = pp.tile([128, len(WNAMES) * 128], MMDT, tag="wpk", name="wpkt")
            nc.sync.dma_start(wpkt[:], wpk[:, :])
            bpkt = pp.tile([128, len(BNAMES)], F32, tag="bpk", name="bpkt")
            nc.sync.dma_start(bpkt[:], bpk[:, :])

            wt = {}
            for i, name in enumerate(WNAMES):
                if name in ("w1eA", "w1eB"):
                    wt[name] = wpkt[0:2 * L, i * 128:(i + 1) * 128]
                elif name in ("dw2A", "dw2B"):
                    wt[name] = wpkt[:, i * 128:i * 128 + 2 * O]
                else:
                    wt[name] = wpkt[:, i * 128:(i + 1) * 128]
            bs = {}
            for j, name in enumerate(BNAMES):
                if name == "db2":
                    bs[name] = bpkt[0:2 * O, j:j + 1]
                else:
                    bs[name] = bpkt[:, j:j + 1]

            # per-pair persistent activations
            field = [pp.tile([128, N], MMDT, tag=f"field{p}", name=f"field{p}")
                     for p in range(PAIRS)]
            state = [pp.tile([128, N], MMDT, tag=f"state{p}", name=f"state{p}")
                     for p in range(PAIRS)]

            # ---- encoder (emitted before the big AT DMA) ----
            for p in range(PAIRS):
                xp = x2p.tile([2 * L, N], MMDT, tag="x2p", name="xp")
                nc.sync.dma_start(xp[0:L, :], hist[2 * p, :, :])
                nc.sync.dma_start(xp[L:2 * L, :], hist[2 * p + 1, :, :])
                hea = hab.tile([128, N], MMDT, tag="hab", name="hea")
                heb = hab.tile([128, N], MMDT, tag="hab", name="heb")
                for (wname, bname, dst) in [("w1eA", "eb1A", hea),
                                            ("w1eB", "eb1B", heb)]:
                    ph = psA.tile([128, N], F32, tag="psA", name="psah")
                    for hf in range(2):
                        sl = slice(hf * 512, (hf + 1) * 512)
                        nc.tensor.matmul(ph[:, sl], wt[wname], xp[:, sl],
                                         start=True, stop=True)
                    nc.scalar.activation(dst[:], ph[:], AF.Relu, bias=bs[bname])
                pf = psB.tile([128, N], F32, tag="psB", name="psbf")
                for hf in range(2):
                    sl = slice(hf * 512, (hf + 1) * 512)
                    nc.tensor.matmul(pf[:, sl], wt["w2eA"], hea[:, sl],
                                     start=True, stop=False)
                    nc.tensor.matmul(pf[:, sl], wt["w2eB"], heb[:, sl],
                                     start=False, stop=True)
                nc.scalar.activation(field[p][:], pf[:], AF.Identity,
                                     bias=bs["eb2"])

            # ---- adjacency operator: host-precomputed, one DMA ----
            AT = pp.tile([128, KCH * N], MMDT, tag="AT", name="AT")
            nc.sync.dma_start(AT[:], ath[:, :])

            # ---- main steps (phase-major software pipelining) ----
            for s in range(STEPS):
                first = (s == 0)
                fts, has, hbs = [], [], []
                for p in range(PAIRS):
                    # A) transpose field pair -> fieldT [m', (k, psi)]
                    ptr = psA.tile([128, N], F32, tag="psA", name="psatr")
                    for k in range(KCH):
                        nc.tensor.matmul(ptr[:, k * 128:(k + 1) * 128],
                                         field[p][:, k * 128:(k + 1) * 128],
                                         wt["ieye"], start=True, stop=True)
                    ft = ftp.tile([128, N], MMDT, tag="ft", name="ft")
                    nc.vector.tensor_copy(ft[:], ptr[:])
                    fts.append(ft)

                    # B) pde layer 1: hA/hB = tanh(field @ w1 + b1)
                    ha = hab.tile([128, N], MMDT, tag="hab", name="ha")
                    hb = hab.tile([128, N], MMDT, tag="hab", name="hb")
                    for (wname, bname, dst) in [("pw1A", "pb1A", ha),
                                                ("pw1B", "pb1B", hb)]:
                        ph = psA.tile([128, N], F32, tag="psA", name="psah")
                        for hf in range(2):
                            sl = slice(hf * 512, (hf + 1) * 512)
                            nc.tensor.matmul(ph[:, sl], wt[wname],
                                             field[p][:, sl],
                                             start=True, stop=True)
                        nc.scalar.activation(dst[:], ph[:], AF.Tanh,
                                             bias=bs[bname])
                    has.append(ha)
                    hbs.append(hb)

                fes = []

                def emit_gru(p, fe_t):
                    z_t = zcp.tile([128, N], MMDT, tag="zc", name="z_t")
                    c_t = zcp.tile([128, N], MMDT, tag="zc", name="c_t")
                    for (wname, uname, bname, func, dst) in [
                        ("wzbd", "uzbd", "bz", AF.Sigmoid, z_t),
                        ("whbd", "uhbd", "bh", AF.Tanh, c_t),
                    ]:
                        pz = psB.tile([128, N], F32, tag="psB", name="psbz")
                        for hf in range(2):
                            sl = slice(hf * 512, (hf + 1) * 512)
                            nc.tensor.matmul(pz[:, sl], wt[wname], fe_t[:, sl],
                                             start=True, stop=first)
                            if not first:
                                nc.tensor.matmul(pz[:, sl], wt[uname],
                                                 state[p][:, sl],
                                                 start=False, stop=True)
                        nc.scalar.activation(dst[:], pz[:], func, bias=bs[bname])
                    if first:
                        nc.vector.tensor_tensor(state[p][:], z_t[:], c_t[:],
                                                ALU.mult)
                    else:
                        t1 = tmp.tile([128, N], MMDT, tag="tmp", name="t1")
                        nc.vector.tensor_tensor(t1[:], c_t[:], state[p][:],
                                                ALU.subtract)
                        nc.vector.tensor_tensor(t1[:], z_t[:], t1[:], ALU.mult)
                        nc.vector.tensor_tensor(state[p][:], state[p][:], t1[:],
                                                ALU.add)

                for p in range(PAIRS):
                    ft, ha, hb = fts[p], has[p], hbs[p]
                    # C) fe psum: Laplacian(-c_lap diag folded) + pde layer 2
                    fe_t = fep.tile([128, N], MMDT, tag="fe", name="fe_t")
                    pfe = psB.tile([128, N], F32, tag="psB", name="psbfe")
                    for hf in range(2):
                        sl = slice(hf * 512, (hf + 1) * 512)
                        for k in range(KCH):
                            nc.tensor.matmul(
                                pfe[:, sl],
                                ft[:, k * 128:(k + 1) * 128],
                                AT[:, k * N + hf * 512:k * N + (hf + 1) * 512],
                                start=(k == 0), stop=False)
                        nc.tensor.matmul(pfe[:, sl], wt["pw2A"], ha[:, sl],
                                         start=False, stop=False)
                        nc.tensor.matmul(pfe[:, sl], wt["pw2B"], hb[:, sl],
                                         start=False, stop=True)
                    # fe = psum + field (pb2 bias folded into bz/bh/bo)
                    nc.vector.tensor_tensor(fe_t[:], pfe[:], field[p][:],
                                            ALU.add)
                    fes.append(fe_t)
                    # D+E) GRU for the previous pair overlaps this Laplacian
                    if p > 0:
                        emit_gru(p - 1, fes[p - 1])
                emit_gru(PAIRS - 1, fes[PAIRS - 1])

                for p in range(PAIRS):
                    # F) field' = fe + state @ wo + bo
                    pf = psB.tile([128, N], F32, tag="psB", name="psbf2")
                    for hf in range(2):
                        sl = slice(hf * 512, (hf + 1) * 512)
                        nc.tensor.matmul(pf[:, sl], wt["wobd"], state[p][:, sl],
                                         start=True, stop=False)
                        nc.tensor.matmul(pf[:, sl], wt["ieye"], fes[p][:, sl],
                                         start=False, stop=True)
                    nc.scalar.activation(field[p][:], pf[:], AF.Identity,
                                         bias=bs["bo"])

            # ---- decoder ----
            for p in range(PAIRS):
                dha = hab.tile([128, N], MMDT, tag="hab", name="dha")
                dhb = hab.tile([128, N], MMDT, tag="hab", name="dhb")
                for (wname, bname, dst) in [("dw1A", "db1A", dha),
                                            ("dw1B", "db1B", dhb)]:
                    ph = psA.tile([128, N], F32, tag="psA", name="psah")
                    for hf in range(2):
                        sl = slice(hf * 512, (hf + 1) * 512)
                        nc.tensor.matmul(ph[:, sl], wt[wname], field[p][:, sl],
                                         start=True, stop=True)
                    nc.scalar.activation(dst[:], ph[:], AF.Relu, bias=bs[bname])
                po = psB.tile([2 * O, N], F32, tag="psB", name="psbo")
                for hf in range(2):
                    sl = slice(hf * 512, (hf + 1) * 512)
                    nc.tensor.matmul(po[:, sl], wt["dw2A"], dha[:, sl],
                                     start=True, stop=False)
                    nc.tensor.matmul(po[:, sl], wt["dw2B"], dhb[:, sl],
                                     start=False, stop=True)
                o2 = o2p.tile([2 * O, N], F32, tag="o2", name="o2")
                nc.scalar.activation(o2[:], po[:], AF.Identity, bias=bs["db2"])
                nc.sync.dma_start(out[2 * p, :, :], o2[0:O, :])
                nc.sync.dma_start(out[2 * p + 1, :, :], o2[O:2 * O, :])

    nc.compile()
    return nc


MMNP = mybir.dt.np(MMDT)


def _blockdiag(w):
    w = np.asarray(w, dtype=np.float64)
    r, c = w.shape
    o = np.zeros((2 * r, 2 * c), dtype=np.float64)
    o[:r, :c] = w
    o[r:, c:] = w
    return o


def _slot(w):
    """place an array into a [128, 128] weight slot."""
    w = np.asarray(w, dtype=np.float64)
    o = np.zeros((128, 128), dtype=np.float64)
    o[:w.shape[0], :w.shape[1]] = w
    return o


def prepare(inputs):
    """Host packing (float64) + compiled Bass module + per-core input maps."""
    g = {k: np.asarray(v) for k, v in inputs.items()}
    pde_mix = float(np.asarray(g["pde_mix"], dtype=np.float64))
    alpha = float(1.0 / (1.0 + np.exp(-pde_mix)))
    dt_ = 1.0 / STEPS
    s2 = (1.0 - alpha) * dt_
    c_lap = alpha * dt_

    f64 = lambda k: np.asarray(g[k], np.float64)
    enc_w1, enc_w2 = f64("enc_w1"), f64("enc_w2")
    pde_w1, pde_w2 = f64("pde_w1"), f64("pde_w2") * s2
    dec_w1, dec_w2 = f64("dec_w1"), f64("dec_w2")

    slots = {
        "w1eA": _blockdiag(enc_w1[:, 0:64]),
        "w1eB": _blockdiag(enc_w1[:, 64:128]),
        "w2eA": _blockdiag(enc_w2[0:64, :]),
        "w2eB": _blockdiag(enc_w2[64:128, :]),
        "pw1A": _blockdiag(pde_w1[:, 0:64]),
        "pw1B": _blockdiag(pde_w1[:, 64:128]),
        "pw2A": _blockdiag(pde_w2[0:64, :]),
        "pw2B": _blockdiag(pde_w2[64:128, :]),
        "wzbd": _blockdiag(f64("ss_wz")),
        "uzbd": _blockdiag(f64("ss_uz")),
        "whbd": _blockdiag(f64("ss_wh")),
        "uhbd": _blockdiag(f64("ss_uh")),
        "wobd": _blockdiag(f64("ss_wo")),
        "dw1A": _blockdiag(dec_w1[:, 0:64]),
        "dw1B": _blockdiag(dec_w1[:, 64:128]),
        "dw2A": _blockdiag(dec_w2[0:64, :]),
        "dw2B": _blockdiag(dec_w2[64:128, :]),
        "ieye": np.eye(128, dtype=np.float64),
    }
    wpk = np.concatenate([_slot(slots[n]) for n in WNAMES], axis=1)

    # biases; pb2 folded into bz/bh/bo (fe carries no bias on device)
    pb2d = f64("pde_b2") * s2
    bz_f = f64("ss_bz") + pb2d @ f64("ss_wz")
    bh_f = f64("ss_bh") + pb2d @ f64("ss_wh")
    bo_f = f64("ss_bo") + pb2d
    bias_vals = {
        "eb1A": np.tile(f64("enc_b1")[0:64], 2),
        "eb1B": np.tile(f64("enc_b1")[64:128], 2),
        "eb2": np.tile(f64("enc_b2"), 2),
        "pb1A": np.tile(f64("pde_b1")[0:64], 2),
        "pb1B": np.tile(f64("pde_b1")[64:128], 2),
        "bz": np.tile(bz_f, 2),
        "bh": np.tile(bh_f, 2),
        "bo": np.tile(bo_f, 2),
        "db1A": np.tile(f64("dec_b1")[0:64], 2),
        "db1B": np.tile(f64("dec_b1")[64:128], 2),
        "db2": np.tile(f64("dec_b2"), 2),
    }
    bpk = np.zeros((128, len(BNAMES)), dtype=np.float64)
    for j, name in enumerate(BNAMES):
        v = bias_vals[name]
        bpk[:len(v), j] = v

    # adjacency operator: softmax rows, scale, subtract diag, transpose
    adj64 = f64("adj")
    e = np.exp(adj64 - adj64.max(axis=-1, keepdims=True))
    A = e / e.sum(axis=-1, keepdims=True)
    M = c_lap * (A - np.eye(N))
    ath = M.T.reshape(KCH, 128, N).transpose(1, 0, 2).reshape(128, KCH * N)

    common = {
        "wpk": np.ascontiguousarray(wpk.astype(np.float32)).astype(MMNP),
        "bpk": np.ascontiguousarray(bpk.astype(np.float32)),
        "ath": np.ascontiguousarray(ath.astype(np.float32)).astype(MMNP),
    }

    hist = np.asarray(g["history_data"], np.float32)[..., 0]  # [B, L, N]
    in_maps = []
    for c in range(NCORES):
        m = dict(common)
        m["hist"] = np.ascontiguousarray(hist[c * BL:(c + 1) * BL]).astype(MMNP)
        in_maps.append(m)

    nc = _build()
    return nc, in_maps


def assemble(results):
    outs = [results[c]["out"] for c in range(NCORES)]          # [BL, O, N]
    full = np.concatenate(outs, axis=0)                        # [B, O, N]
    return np.ascontiguousarray(full[..., None].astype(np.float32))


def kernel(**inputs) -> np.ndarray:
    nc, in_maps = prepare(inputs)
    res = run_bass_kernel_spmd(nc, in_maps, core_ids=list(range(NCORES)))
    return assemble(res.results)


# revision 13
# speedup vs baseline: 1.7896x; 1.0020x over previous
"""Trainium2 Bass kernel for nn_CLFMv2_NoTemporalEmb (graph-PDE message passing).

Strategy: data-parallel over batch B=64 across 8 NeuronCores (8 batches/core).
Per core, activations are "pair-packed feature-major":
    tensor[psi, n],  psi = (batch_parity)*64 + d  (128 partitions),
    one [128, 1024] tensor per batch-pair (4 pairs/core).
Weight matmuls use block-diagonal [128,128] stationary operands so K=128,
M=128, PSUM dst partition 0. The Laplacian A@field uses PE-transposed field
tiles (regular matmuls against identity so HAM stays warm) as stationary
operands against the host-precomputed alpha*dt*(softmax(adj) - I) transpose;
the softmax and all weight packing run on host in float64.
Matmuls run in bf16 (full PE rate); PSUM accumulates fp32.
"""

import os
import contextlib

import numpy as np

import concourse.bacc as bacc
import concourse.tile as tile
import concourse.mybir as mybir
from concourse.bass_utils import run_bass_kernel_spmd

F32 = mybir.dt.float32
F32R = mybir.dt.float32r
BF16 = mybir.dt.bfloat16
MMDT = F32R if os.environ.get("KMM_DTYPE", "bf16") == "f32r" else BF16
AF = mybir.ActivationFunctionType
ALU = mybir.AluOpType

B, L, N, D, H, O = 64, 12, 1024, 64, 128, 12
STEPS = 4
NCORES = 8
BL = B // NCORES          # 8 batches per core
PAIRS = BL // 2           # 4
KCH = N // 128            # 8 adjacency chunks

# weight-pack slot order (each slot is a [128, 128] block in wpk)
WNAMES = ["w1eA", "w1eB", "w2eA", "w2eB", "pw1A", "pw1B", "pw2A", "pw2B",
          "wzbd", "uzbd", "whbd", "uhbd", "wobd", "dw1A", "dw1B",
          "dw2A", "dw2B", "ieye"]
BNAMES = ["eb1A", "eb1B", "eb2", "pb1A", "pb1B", "bz", "bh", "bo",
          "db1A", "db1B", "db2"]


def _build():
    nc = bacc.Bacc("TRN2", target_bir_lowering=False, debug=False)

    wpk = nc.dram_tensor("wpk", [128, len(WNAMES) * 128], MMDT,
                         kind="ExternalInput")
    bpk = nc.dram_tensor("bpk", [128, len(BNAMES)], F32, kind="ExternalInput")
    hist = nc.dram_tensor("hist", [BL, L, N], MMDT, kind="ExternalInput")
    ath = nc.dram_tensor("ath", [128, KCH * N], MMDT, kind="ExternalInput")
    out = nc.dram_tensor("out", [BL, O, N], F32, kind="ExternalOutput")

    with tile.TileContext(nc) as tc:
        with contextlib.ExitStack() as ctx:
            pp = ctx.enter_context(tc.tile_pool(name="persist", bufs=1))
            hab = ctx.enter_context(tc.tile_pool(name="hab", bufs=10))
            ftp = ctx.enter_context(tc.tile_pool(name="ftp", bufs=5))
            tmp = ctx.enter_context(tc.tile_pool(name="tmp", bufs=2))
            fep = ctx.enter_context(tc.tile_pool(name="fep", bufs=5))
            zcp = ctx.enter_context(tc.tile_pool(name="zcp", bufs=4))
            x2p = ctx.enter_context(tc.tile_pool(name="x2p", bufs=2))
            o2p = ctx.enter_context(tc.tile_pool(name="o2p", bufs=2))
            psA = ctx.enter_context(tc.tile_pool(name="psA", bufs=2, space="PSUM"))
            psB = ctx.enter_context(tc.tile_pool(name="psB", bufs=2, space="PSUM"))

            # ---- packed weights and biases: two DMAs ----
            wpkt = pp.tile([128, len(WNAMES) * 128], MMDT, tag="wpk", name="wpkt")
            nc.sync.dma_start(wpkt[:], wpk[:, :])
            bpkt = pp.tile([128, len(BNAMES)], F32, tag="bpk", name="bpkt")
            nc.sync.dma_start(bpkt[:], bpk[:, :])

            wt = {}
            for i, name in enumerate(WNAMES):
                if name in ("w1eA", "w1eB"):
                    wt[name] = wpkt[0:2 * L, i * 128:(i + 1) * 128]
                elif name in ("dw2A", "dw2B"):
                    wt[name] = wpkt[:, i * 128:i * 128 + 2 * O]
                else:
                    wt[name] = wpkt[:, i * 128:(i + 1) * 128]
            bs = {}
            for j, name in enumerate(BNAMES):
                if name == "db2":
                    bs[name] = bpkt[0:2 * O, j:j + 1]
                else:
                    bs[name] = bpkt[:, j:j + 1]

            # per-pair persistent activations
            field = [pp.tile([128, N], MMDT, tag=f"field{p}", name=f"field{p}")
                     for p in range(PAIRS)]
            state = [pp.tile([128, N], MMDT, tag=f"state{p}", name=f"state{p}")
                     for p in range(PAIRS)]

            # ---- encoder (emitted before the big AT DMA) ----
            for p in range(PAIRS):
                xp = x2p.tile([2 * L, N], MMDT, tag="x2p", name="xp")
                nc.sync.dma_start(xp[0:L, :], hist[2 * p, :, :])
                nc.sync.dma_start(xp[L:2 * L, :], hist[2 * p + 1, :, :])
                hea = hab.tile([128, N], MMDT, tag="hab", name="hea")
                heb = hab.tile([128, N], MMDT, tag="hab", name="heb")
                for (wname, bname, dst) in [("w1eA", "eb1A", hea),
                                            ("w1eB", "eb1B", heb)]:
                    ph = psA.tile([128, N], F32, tag="psA", name="psah")
                    for hf in range(2):
                        sl = slice(hf * 512, (hf + 1) * 512)
                        nc.tensor.matmul(ph[:, sl], wt[wname], xp[:, sl],
                                         start=True, stop=True)
                    nc.scalar.activation(dst[:], ph[:], AF.Relu, bias=bs[bname])
                pf = psB.tile([128, N], F32, tag="psB", name="psbf")
                for hf in range(2):
                    sl = slice(hf * 512, (hf + 1) * 512)
                    nc.tensor.matmul(pf[:, sl], wt["w2eA"], hea[:, sl],
                                     start=True, stop=False)
                    nc.tensor.matmul(pf[:, sl], wt["w2eB"], heb[:, sl],
                                     start=False, stop=True)
                nc.scalar.activation(field[p][:], pf[:], AF.Identity,
                                     bias=bs["eb2"])

            # ---- adjacency operator: host-precomputed, one DMA ----
            AT = pp.tile([128, KCH * N], MMDT, tag="AT", name="AT")
            nc.sync.dma_start(AT[:], ath[:, :])

            # ---- main steps (phase-major software pipelining) ----
            for s in range(STEPS):
                first = (s == 0)
                fts, has, hbs = [], [], []
                for p in range(PAIRS):
                    # A) transpose field pair -> fieldT [m', (k, psi)]
                    ptr = psA.tile([128, N], F32, tag="psA", name="psatr")
                    for k in range(KCH):
                        nc.tensor.matmul(ptr[:, k * 128:(k + 1) * 128],
                                         field[p][:, k * 128:(k + 1) * 128],
                                         wt["ieye"], start=True, stop=True)
                    ft = ftp.tile([128, N], MMDT, tag="ft", name="ft")
                    nc.vector.tensor_copy(ft[:], ptr[:])
                    fts.append(ft)

                    # B) pde layer 1: hA/hB = tanh(field @ w1 + b1)
                    ha = hab.tile([128, N], MMDT, tag="hab", name="ha")
                    hb = hab.tile([128, N], MMDT, tag="hab", name="hb")
                    for (wname, bname, dst) in [("pw1A", "pb1A", ha),
                                                ("pw1B", "pb1B", hb)]:
                        ph = psA.tile([128, N], F32, tag="psA", name="psah")
                        for hf in range(2):
                            sl = slice(hf * 512, (hf + 1) * 512)
                            nc.tensor.matmul(ph[:, sl], wt[wname],
                                             field[p][:, sl],
                                             start=True, stop=True)
                        nc.scalar.activation(dst[:], ph[:], AF.Tanh,
                                             bias=bs[bname])
                    has.append(ha)
                    hbs.append(hb)

                fes = []

                def emit_gru(p, fe_t):
                    z_t = zcp.tile([128, N], MMDT, tag="zc", name="z_t")
                    c_t = zcp.tile([128, N], MMDT, tag="zc", name="c_t")
                    for (wname, uname, bname, func, dst) in [
                        ("wzbd", "uzbd", "bz", AF.Sigmoid, z_t),
                        ("whbd", "uhbd", "bh", AF.Tanh, c_t),
                    ]:
                        pz = psB.tile([128, N], F32, tag="psB", name="psbz")
                        for hf in range(2):
                            sl = slice(hf * 512, (hf + 1) * 512)
                            nc.tensor.matmul(pz[:, sl], wt[wname], fe_t[:, sl],
                                             start=True, stop=first)
                            if not first:
                                nc.tensor.matmul(pz[:, sl], wt[uname],
                                                 state[p][:, sl],
                                                 start=False, stop=True)
                        nc.scalar.activation(dst[:], pz[:], func, bias=bs[bname])
                    if first:
                        nc.vector.tensor_tensor(state[p][:], z_t[:], c_t[:],
                                                ALU.mult)
                    else:
                        t1 = tmp.tile([128, N], MMDT, tag="tmp", name="t1")
                        nc.vector.tensor_tensor(t1[:], c_t[:], state[p][:],
                                                ALU.subtract)
                        nc.vector.tensor_tensor(t1[:], z_t[:], t1[:], ALU.mult)
                        nc.vector.tensor_tensor(state[p][:], state[p][:], t1[:],
                                                ALU.add)

                for p in range(PAIRS):
                    ft, ha, hb = fts[p], has[p], hbs[p]
                    # C) fe psum: Laplacian(-c_lap diag folded) + pde layer 2
                    fe_t = fep.tile([128, N], MMDT, tag="fe", name="fe_t")
                    pfe = psB.tile([128, N], F32, tag="psB", name="psbfe")
                    for hf in range(2):
                        sl = slice(hf * 512, (hf + 1) * 512)
                        for k in range(KCH):
                            nc.tensor.matmul(
                                pfe[:, sl],
                                ft[:, k * 128:(k + 1) * 128],
                                AT[:, k * N + hf * 512:k * N + (hf + 1) * 512],
                                start=(k == 0), stop=False)
                        nc.tensor.matmul(pfe[:, sl], wt["pw2A"], ha[:, sl],
                                         start=False, stop=False)
                        nc.tensor.matmul(pfe[:, sl], wt["pw2B"], hb[:, sl],
                                         start=False, stop=True)
                    # fe = psum + field (pb2 bias folded into bz/bh/bo)
                    nc.vector.tensor_tensor(fe_t[:], pfe[:], field[p][:],
                                            ALU.add)
                    fes.append(fe_t)
                    # D+E) GRU trails the Laplacian by two pairs for slack
                    if p >= 2:
                        emit_gru(p - 2, fes[p - 2])
                emit_gru(PAIRS - 2, fes[PAIRS - 2])
                emit_gru(PAIRS - 1, fes[PAIRS - 1])

                for p in range(PAIRS):
                    # F) field' = fe + state @ wo + bo
                    pf = psB.tile([128, N], F32, tag="psB", name="psbf2")
                    for hf in range(2):
                        sl = slice(hf * 512, (hf + 1) * 512)
                        nc.tensor.matmul(pf[:, sl], wt["wobd"], state[p][:, sl],
                                         start=True, stop=False)
                        nc.tensor.matmul(pf[:, sl], wt["ieye"], fes[p][:, sl],
                                         start=False, stop=True)
                    nc.scalar.activation(field[p][:], pf[:], AF.Identity,
                                         bias=bs["bo"])

            # ---- decoder ----
            for p in range(PAIRS):
                dha = hab.tile([128, N], MMDT, tag="hab", name="dha")
                dhb = hab.tile([128, N], MMDT, tag="hab", name="dhb")
                for (wname, bname, dst) in [("dw1A", "db1A", dha),
                                            ("dw1B", "db1B", dhb)]:
                    ph = psA.tile([128, N], F32, tag="psA", name="psah")
                    for hf in range(2):
                        sl = slice(hf * 512, (hf + 1) * 512)
                        nc.tensor.matmul(ph[:, sl], wt[wname], field[p][:, sl],
                                         start=True, stop=True)
                    nc.scalar.activation(dst[:], ph[:], AF.Relu, bias=bs[bname])
                po = psB.tile([2 * O, N], F32, tag="psB", name="psbo")
                for hf in range(2):
                    sl = slice(hf * 512, (hf + 1) * 512)
                    nc.tensor.matmul(po[:, sl], wt["dw2A"], dha[:, sl],
                                     start=True, stop=False)
                    nc.tensor.matmul(po[:, sl], wt["dw2B"], dhb[:, sl],
                                     start=False, stop=True)
                o2 = o2p.tile([2 * O, N], F32, tag="o2", name="o2")
                nc.scalar.activation(o2[:], po[:], AF.Identity, bias=bs["db2"])
                nc.sync.dma_start(out[2 * p, :, :], o2[0:O, :])
                nc.sync.dma_start(out[2 * p + 1, :, :], o2[O:2 * O, :])

    nc.compile()
    return nc


MMNP = mybir.dt.np(MMDT)


def _blockdiag(w):
    w = np.asarray(w, dtype=np.float64)
    r, c = w.shape
    o = np.zeros((2 * r, 2 * c), dtype=np.float64)
    o[:r, :c] = w
    o[r:, c:] = w
    return o


def _slot(w):
    """place an array into a [128, 128] weight slot."""
    w = np.asarray(w, dtype=np.float64)
    o = np.zeros((128, 128), dtype=np.float64)
    o[:w.shape[0], :w.shape[1]] = w
    return o


def prepare(inputs):
    """Host packing (float64) + compiled Bass module + per-core input maps."""
    g = {k: np.asarray(v) for k, v in inputs.items()}
    pde_mix = float(np.asarray(g["pde_mix"], dtype=np.float64))
    alpha = float(1.0 / (1.0 + np.exp(-pde_mix)))
    dt_ = 1.0 / STEPS
    s2 = (1.0 - alpha) * dt_
    c_lap = alpha * dt_

    f64 = lambda k: np.asarray(g[k], np.float64)
    enc_w1, enc_w2 = f64("enc_w1"), f64("enc_w2")
    pde_w1, pde_w2 = f64("pde_w1"), f64("pde_w2") * s2
    dec_w1, dec_w2 = f64("dec_w1"), f64("dec_w2")

    slots = {
        "w1eA": _blockdiag(enc_w1[:, 0:64]),
        "w1eB": _blockdiag(enc_w1[:, 64:128]),
        "w2eA": _blockdiag(enc_w2[0:64, :]),
        "w2eB": _blockdiag(enc_w2[64:128, :]),
        "pw1A": _blockdiag(pde_w1[:, 0:64]),
        "pw1B": _blockdiag(pde_w1[:, 64:128]),
        "pw2A": _blockdiag(pde_w2[0:64, :]),
        "pw2B": _blockdiag(pde_w2[64:128, :]),
        "wzbd": _blockdiag(f64("ss_wz")),
        "uzbd": _blockdiag(f64("ss_uz")),
        "whbd": _blockdiag(f64("ss_wh")),
        "uhbd": _blockdiag(f64("ss_uh")),
        "wobd": _blockdiag(f64("ss_wo")),
        "dw1A": _blockdiag(dec_w1[:, 0:64]),
        "dw1B": _blockdiag(dec_w1[:, 64:128]),
        "dw2A": _blockdiag(dec_w2[0:64, :]),
        "dw2B": _blockdiag(dec_w2[64:128, :]),
        "ieye": np.eye(128, dtype=np.float64),
    }
    wpk = np.concatenate([_slot(slots[n]) for n in WNAMES], axis=1)

    # biases; pb2 folded into bz/bh/bo (fe carries no bias on device)
    pb2d = f64("pde_b2") * s2
    bz_f = f64("ss_bz") + pb2d @ f64("ss_wz")
    bh_f = f64("ss_bh") + pb2d @ f64("ss_wh")
    bo_f = f64("ss_bo") + pb2d
    bias_vals = {
        "eb1A": np.tile(f64("enc_b1")[0:64], 2),
        "eb1B": np.tile(f64("enc_b1")[64:128], 2),
        "eb2": np.tile(f64("enc_b2"), 2),
        "pb1A": np.tile(f64("pde_b1")[0:64], 2),
        "pb1B": np.tile(f64("pde_b1")[64:128], 2),
        "bz": np.tile(bz_f, 2),
        "bh": np.tile(bh_f, 2),
        "bo": np.tile(bo_f, 2),
        "db1A": np.tile(f64("dec_b1")[0:64], 2),
        "db1B": np.tile(f64("dec_b1")[64:128], 2),
        "db2": np.tile(f64("dec_b2"), 2),
    }
    bpk = np.zeros((128, len(BNAMES)), dtype=np.float64)
    for j, name in enumerate(BNAMES):
        v = bias_vals[name]
        bpk[:len(v), j] = v

    # adjacency operator: softmax rows, scale, subtract diag, transpose
    adj64 = f64("adj")
    e = np.exp(adj64 - adj64.max(axis=-1, keepdims=True))
    A = e / e.sum(axis=-1, keepdims=True)
    M = c_lap * (A - np.eye(N))
    ath = M.T.reshape(KCH, 128, N).transpose(1, 0, 2).reshape(128, KCH * N)

    common = {
        "wpk": np.ascontiguousarray(wpk.astype(np.float32)).astype(MMNP),
        "bpk": np.ascontiguousarray(bpk.astype(np.float32)),
        "ath": np.ascontiguousarray(ath.astype(np.float32)).astype(MMNP),
    }

    hist = np.asarray(g["history_data"], np.float32)[..., 0]  # [B, L, N]
    in_maps = []
    for c in range(NCORES):
        m = dict(common)
        m["hist"] = np.ascontiguousarray(hist[c * BL:(c + 1) * BL]).astype(MMNP)
        in_maps.append(m)

    nc = _build()
    return nc, in_maps


def assemble(results):
    outs = [results[c]["out"] for c in range(NCORES)]          # [BL, O, N]
    full = np.concatenate(outs, axis=0)                        # [B, O, N]
    return np.ascontiguousarray(full[..., None].astype(np.float32))


def kernel(**inputs) -> np.ndarray:
    nc, in_maps = prepare(inputs)
    res = run_bass_kernel_spmd(nc, in_maps, core_ids=list(range(NCORES)))
    return assemble(res.results)
